# revision 14
# baseline (speedup 1.0000x reference)
"""Trainium2 Bass kernel for KV-cached (causal) multi-head attention.

Full module: y = softmax(mask(QK^T/sqrt(hd))) V  -> out_proj, with
Q/K/V = linear projections of query/key/value inputs.

Shapes (hardcoded): B=2, S=2048, D=2048, H=16 heads, hd=128.

Sharding (8 NeuronCores): core c handles batch b=c//4 and head group
g=c%4 (4 heads = 512 dims).  The host<->device tunnel is the wall-time
bottleneck (~50-80 MB/s shared pipe), so all bulk traffic is int8:

  - activations: core c receives the c%4-th 512-row slice of its
    batch's query/key/value, transposed to [D, 512] and int8-quantized
    per feature row (scale = absmax/127 over the 512 seq positions).
    An AllGather over the batch group {4b..4b+3} rebuilds the full
    [D, S] transposed activations as 4 column blocks.
  - weights: int8 with per-contraction-row scales taken over the FULL
    output row (so the scale is head-group independent); cores c and
    c+4 need identical TP weight slices, so each receives half and an
    AllGather over pairs {c, c+4} rebuilds them.  Because both the
    activation scale and the weight scale are per-contraction-dim, the
    host pre-multiplies them into a single combined scale applied to
    the activation tiles on device; the int8 weights are then used
    EXACTLY (integers <=127 are exact in fp16/bf16).
  - the V bias is folded into the output bias on the host
    (softmax rows sum to 1, so P(V + bv) = PV + bv after
    normalization => bo' = bo + bv @ Wo.T), removing it from the
    device entirely.
  - output: partial out-projections are summed on device with a
    ReduceScatter over the batch group; each core returns only its
    512-row slice of y, int8-quantized with per-row f32 scales that
    are bit-packed into 4 extra int8 columns (single output tensor).

On-device layout (fp16 matmuls on the Q/K path for extra mantissa,
bf16 elsewhere, fp32 PSUM accumulation):
  - Q^T, K^T computed as [dq, S] (head dim on partitions) so that
    scores = Q^T.T @ K^T needs no on-device transposes
  - V computed as [S, dv]
  - softmax per q-row (partition) along free kv axis; exp on ScalarE
    with fused per-chunk row-sums (accum_out); causal handled by
    skipping kv blocks beyond the diagonal + one additive mask tile
    on the diagonal 128x128 block
  - P^T for the PV matmul via PE-mode transposes of 128x128 blocks
  - attention output [q, hd] re-transposed per 128-block to feed the
    output projection as lhsT
"""

import sys

for _p in ("/opt/trn_rl_repo",):
    if _p not in sys.path:
        sys.path.insert(0, _p)

from contextlib import ExitStack

import numpy as np

import concourse.bass as bass
import concourse.mybir as mybir
import concourse.tile as tile
from concourse.vector_clock import ScopedClock
from concourse.masks import make_causal_mask, make_identity

BF16 = mybir.dt.bfloat16
FP16 = mybir.dt.float16
F32 = mybir.dt.float32
I8 = mybir.dt.int8

B, S, D = 2, 2048, 2048
NH, HD = 16, 128          # total heads, head dim
GH = 4                    # heads per core
GD = GH * HD              # 512 dims per core
P = 128
SCALE = 1.0 / np.sqrt(HD)
N_CORES = 8

GROUPS_BATCH = [[0, 1, 2, 3], [4, 5, 6, 7]]   # share one batch's acts
GROUPS_PAIR = [[0, 4], [1, 5], [2, 6], [3, 7]]  # share TP weight slices

# Single packed int8 input, [XW_ROWS, 512] rows per core:
#   rows 0..6143     : qT/kT/vT activation slices (2048 rows each)
#   rows 6144..6271  : f32 scale/bias block bit-packed into int8 (240 of
#                      512 bytes per row used); AllGathered with the acts
#   rows 6272..9343  : wq/wk/wv int8 half-slices (1024 rows each)
#   rows 9344..10367 : wo int8 half, packed as 8 [128, 512] tiles
#                      (lc, oc) so gathered tiles stay partition-friendly
# scl columns: 0..47 combined act*weight scales (j*16+kc), 48..51 wo row
# scales (full 512 of this group as 4 chunks), 52..55 bq*SCALE, 56..59 bk
R_ACTS = 3 * D                   # 6144
BATCH_BLK = R_ACTS + P           # 6272 rows gathered over the batch group
R_W = BATCH_BLK                  # weight half rows start here
XW_ROWS = BATCH_BLK + 3 * (D // 2) + (D // 2)   # 10368
SCL_C = 60


def _drain_and_barrier_split(self, tick_clock, wait_clock):
    # The walrus build in this container rejects a Drain carrying more
    # than one sync wait ("Too many sync wait commands").  Semantically
    # equivalent: chain one drain per wait on the sync engine.
    nc = self.nc
    drain_inst = nc.sync.drain()
    wait_clock.add_sem_waits(
        drain_inst.ins, ScopedClock({None: tick_clock.global_clock})
    )
    si = drain_inst.ins.sync_info
    waits = list(si.on_wait)
    if len(waits) > 1:
        drain_inst.ins.sync_info = mybir.SyncInfo(
            on_wait=[waits[0]], on_update=list(si.on_update)
        )
        for w in waits[1:]:
            d = nc.sync.drain()
            d.ins.sync_info = mybir.SyncInfo(on_wait=[w], on_update=[])
    nc.all_engine_barrier()
    assert self.sems is not None
    popped = nc._tile_sem_poison_stack.pop()
    assert popped is self._sem_poison
    nc.clear_and_free_semaphores(list(self.sems.allocated().values()))
    nc.all_engine_barrier()


tile.TileContext._drain_and_barrier = _drain_and_barrier_split


def _split_multi_waits(nc, max_waits=1):
    """This container's walrus rejects instructions carrying more than one
    sync wait.  Hoist extra waits onto same-engine NoOps placed just before
    the instruction (waits execute in engine program order, so this is
    semantically identical)."""
    uid = [0]
    for fn in nc.m.functions:
        for bb in fn.blocks:
            insts = bb.instructions
            new = []
            changed = False
            for inst in insts:
                si = getattr(inst, "sync_info", None)
                waits = list(si.on_wait) if si is not None else []
                if len(waits) > max_waits:
                    changed = True
                    n_keep = max_waits
                    for w in waits[:-n_keep]:
                        nop = mybir.InstNoOp(
                            name=f"WSPLIT-{uid[0]}", ins=[], outs=[]
                        )
                        uid[0] += 1
                        nop.engine = inst.engine
                        nop.sync_info = mybir.SyncInfo(
                            on_wait=[w], on_update=[]
                        )
                        new.append(nop)
                    inst.sync_info = mybir.SyncInfo(
                        on_wait=waits[-n_keep:], on_update=list(si.on_update)
                    )
                new.append(inst)
            if changed:
                bb.instructions = new
    return nc


def build_bass():
    nc = bass.Bass(num_devices=N_CORES)
    xw_in = nc.declare_dram_parameter("xw", [XW_ROWS, GD], I8, isOutput=False)
    # full batch-major int8 y (+bit-packed f32 row scales), identical on
    # every core via a final AllGather so the host fetches ONE replica
    y = nc.declare_dram_parameter("y", [B * S, D + 4], I8, isOutput=True)

    KC = D // P               # 16 contraction chunks of 128
    TT = S // 512             # 4 t-tiles of 512
    QI = S // P               # 16 q tiles of 128

    with tile.TileContext(nc) as tc, ExitStack() as ctx:
        # ---- DRAM staging + collectives ----
        dram = ctx.enter_context(tc.tile_pool(name="dram", bufs=1, space="DRAM"))
        xw_loc = dram.tile([XW_ROWS, GD], I8, tag="xw_loc")
        xs_g = dram.tile([4 * BATCH_BLK, GD], I8, tag="xs_g")
        w_g = dram.tile([2 * (XW_ROWS - R_W), GD], I8, tag="w_g", name="w_g")
        y_part = dram.tile([S, D], F32, tag="y_part")
        y_red = dram.tile([GD, D], F32, tag="y_red")

        nc.sync.dma_start(xw_loc[:], xw_in[:])
        nc.gpsimd.collective_compute(
            "AllGather", mybir.AluOpType.bypass,
            replica_groups=GROUPS_BATCH,
            ins=[xw_loc[0:BATCH_BLK, :].opt()], outs=[xs_g.opt()],
        )
        nc.gpsimd.collective_compute(
            "AllGather", mybir.AluOpType.bypass,
            replica_groups=GROUPS_PAIR,
            ins=[xw_loc[R_W:XW_ROWS, :].opt()], outs=[w_g.opt()],
        )
        # weight-half h (0/1) of the pair lives at w_g rows h*4096..
        WH = XW_ROWS - R_W        # 4096 rows per half

        const = ctx.enter_context(tc.tile_pool(name="const", bufs=1))
        maskt = const.tile([P, P], F32)
        make_causal_mask(nc, maskt, mask_val=-1e9)
        ident = const.tile([P, P], BF16)
        make_identity(nc, ident)
        scl_sb = const.tile([P, SCL_C], F32)
        nc.sync.dma_start(
            scl_sb[:], xw_loc[R_ACTS:R_ACTS + P, 0:4 * SCL_C].bitcast(F32)
        )
        sclg_sb = []
        for tt in range(TT):
            t = const.tile([P, SCL_C], F32, tag=f"sclg{tt}")
            r = tt * BATCH_BLK + R_ACTS
            nc.sync.dma_start(t[:], xs_g[r:r + P, 0:4 * SCL_C].bitcast(F32))
            sclg_sb.append(t)
        bq_sb = scl_sb[:, 52:52 + GH]
        bk_sb = scl_sb[:, 56:56 + GH]

        # resident weights: int8 converted to fp16 (q/k) / bf16 (v, wo);
        # integer values are exact in 16-bit floats.
        wpool = ctx.enter_context(tc.tile_pool(name="weights", bufs=1))
        wst = ctx.enter_context(tc.tile_pool(name="wst", bufs=6))
        wq_sb, wk_sb, wv_sb = [], [], []
        for i, (name, lst, dt_) in enumerate((
            ("wq", wq_sb, FP16),
            ("wk", wk_sb, FP16),
            ("wv", wv_sb, BF16),
        )):
            for kc in range(KC):
                st = wst.tile([P, GD], I8, tag="wst")
                r = (kc // 8) * WH + i * (D // 2) + (kc % 8) * P
                nc.sync.dma_start(st[:], w_g[r:r + P, :])
                t = wpool.tile([P, GD], dt_, name=f"{name}{kc}", tag=f"{name}{kc}")
                nc.vector.tensor_copy(t[:], st[:])
                lst.append(t)
        # wo: 16 tiles [128, 512] indexed [hb][oc]; gathered tile (hb, oc)
        # sits at w_g row (hb//2)*WH + 3072 + ((hb%2)*4 + oc)*128
        wo_sb = []
        for hb in range(GH):
            row = []
            for oc in range(TT):
                st = wst.tile([P, GD], I8, tag="wst")
                r = (hb // 2) * WH + 3 * (D // 2) + ((hb % 2) * 4 + oc) * P
                nc.sync.dma_start(st[:], w_g[r:r + P, :])
                t = wpool.tile([P, GD], BF16, name=f"woc{hb}_{oc}",
                               tag=f"wo{hb}_{oc}")
                nc.scalar.activation(
                    t[:], st[:], mybir.ActivationFunctionType.Identity,
                    scale=scl_sb[:, 48 + hb:49 + hb],
                )
                row.append(t)
            wo_sb.append(row)

        # persistent activations
        act = ctx.enter_context(tc.tile_pool(name="acts", bufs=1))
        qT_sb = [act.tile([P, S], FP16, name=f"qT{h}", tag=f"qT{h}") for h in range(GH)]
        kT_sb = [act.tile([P, S], FP16, name=f"kT{h}", tag=f"kT{h}") for h in range(GH)]
        v_sb = [act.tile([P, GD], BF16, name=f"v{i}", tag=f"v{i}") for i in range(QI)]

        ctxA = ExitStack()
        xin = ctxA.enter_context(tc.tile_pool(name="xin", bufs=24))
        xdq = ctxA.enter_context(tc.tile_pool(name="xdq", bufs=24))
        ps512 = ctx.enter_context(
            tc.tile_pool(name="ps512", bufs=4, space="PSUM")
        )

        # xs_g row offset for (column-block tt, tensor j, contraction chunk kc)
        def _xrow(tt, j, kc):
            return tt * BATCH_BLK + j * D + kc * P

        # ---- Q^T / K^T projections: out [dq=512, S] in fp16 ----
        for j, (w_sb, out_tiles, b_tile, scale) in enumerate((
            (wq_sb, qT_sb, bq_sb, SCALE),
            (wk_sb, kT_sb, bk_sb, 1.0),
        )):
            for tt in range(TT):
                xch = []
                for kc in range(KC):
                    ti = xin.tile([P, 512], I8, tag="xin")
                    r = _xrow(tt, j, kc)
                    nc.sync.dma_start(ti[:], xs_g[r:r + P, :])
                    td = xdq.tile([P, 512], FP16, tag="xdq")
                    c = j * KC + kc
                    nc.scalar.activation(
                        td[:], ti[:], mybir.ActivationFunctionType.Identity,
                        scale=sclg_sb[tt][:, c:c + 1],
                    )
                    xch.append(td)
                for dt in range(GH):
                    ps = ps512.tile([P, 512], F32, tag="ps512")
                    for kc in range(KC):
                        nc.tensor.matmul(
                            ps[:],
                            lhsT=w_sb[kc][:, dt * P:(dt + 1) * P],
                            rhs=xch[kc][:],
                            start=(kc == 0),
                            stop=(kc == KC - 1),
                        )
                    # evict: out = (psum + b) * scale, bias pre-scaled on host
                    nc.scalar.activation(
                        out_tiles[dt][:, tt * 512:(tt + 1) * 512],
                        ps[:],
                        mybir.ActivationFunctionType.Identity,
                        bias=b_tile[:, dt:dt + 1],
                        scale=scale,
                    )

        # ---- V projection: out [S, dv=512] in bf16, no bias (folded) ----
        for ttg in range(TT):
            xch = []
            for kc in range(KC):
                ti = xin.tile([P, 512], I8, tag="xin")
                r = _xrow(ttg, 2, kc)
                nc.sync.dma_start(ti[:], xs_g[r:r + P, :])
                td = xdq.tile([P, 512], BF16, tag="xdq")
                c = 2 * KC + kc
                nc.scalar.activation(
                    td[:], ti[:], mybir.ActivationFunctionType.Identity,
                    scale=sclg_sb[ttg][:, c:c + 1],
                )
                xch.append(td)
            for sub in range(4):
                ps = ps512.tile([P, 512], F32, tag="ps512")
                for kc in range(KC):
                    nc.tensor.matmul(
                        ps[:],
                        lhsT=xch[kc][:, sub * P:(sub + 1) * P],
                        rhs=wv_sb[kc][:],
                        start=(kc == 0),
                        stop=(kc == KC - 1),
                    )
                nc.vector.tensor_copy(v_sb[ttg * 4 + sub][:], ps[:])

        ctxA.close()

        # ---- attention + output projection, per q tile ----
        ppool = ctx.enter_context(tc.tile_pool(name="p", bufs=2))
        spool = ctx.enter_context(tc.tile_pool(name="sums", bufs=8))
        ps_t = ctx.enter_context(tc.tile_pool(name="ps_t", bufs=2, space="PSUM"))
        ps_o = ctx.enter_context(tc.tile_pool(name="ps_o", bufs=2, space="PSUM"))
        ptp_pool = ctx.enter_context(tc.tile_pool(name="pt", bufs=3))
        at_pool = ctx.enter_context(tc.tile_pool(name="at", bufs=5))
        attn_pool = ctx.enter_context(tc.tile_pool(name="attn", bufs=2))
        ypool = ctx.enter_context(tc.tile_pool(name="ysb", bufs=3))

        for qi in range(QI):
            kv_len = (qi + 1) * P
            nchunks = (kv_len + 511) // 512
            attn_t = attn_pool.tile([P, GD], BF16, tag="attn")
            for h in range(GH):
                p_t = ppool.tile([P, S], BF16, tag="p")
                sums = spool.tile([P, 4], F32, tag="sums")
                for c in range(nchunks):
                    n = min(512, kv_len - c * 512)
                    ps = ps512.tile([P, 512], F32, tag="ps512")
                    nc.tensor.matmul(
                        ps[:, :n],
                        lhsT=qT_sb[h][:, qi * P:(qi + 1) * P],
                        rhs=kT_sb[h][:, c * 512:c * 512 + n],
                        start=True,
                        stop=True,
                    )
                    if c == nchunks - 1:
                        nc.vector.tensor_add(
                            ps[:, n - P:n], ps[:, n - P:n], maskt[:]
                        )
                    nc.scalar.activation(
                        p_t[:, c * 512:c * 512 + n],
                        ps[:, :n],
                        mybir.ActivationFunctionType.Exp,
                        accum_out=sums[:, c:c + 1],
                    )
                tot = spool.tile([P, 1], F32, tag="tot")
                nc.vector.reduce_sum(
                    tot[:], sums[:, :nchunks], axis=mybir.AxisListType.X
                )
                rec = spool.tile([P, 1], F32, tag="rec")
                nc.vector.reciprocal(rec[:], tot[:])

                po = ps_o.tile([P, P], F32)
                pts = {}

                def _pv_transpose(kb):
                    ptp = ps_t.tile([P, P], BF16, tag="ptp")
                    nc.tensor.transpose(
                        ptp[:], p_t[:, kb * P:(kb + 1) * P], ident[:]
                    )
                    s = ptp_pool.tile([P, P], BF16, tag="pt")
                    nc.vector.tensor_copy(s[:], ptp[:])
                    pts[kb] = s

                # pipeline transposes one block ahead of the PV matmuls so
                # the PE never waits on the DVE copy of the current block
                _pv_transpose(0)
                for kb in range(qi + 1):
                    if kb + 1 <= qi:
                        _pv_transpose(kb + 1)
                    nc.tensor.matmul(
                        po[:],
                        lhsT=pts.pop(kb)[:],
                        rhs=v_sb[kb][:, h * P:(h + 1) * P],
                        start=(kb == 0),
                        stop=(kb == qi),
                    )
                nc.vector.tensor_scalar_mul(
                    attn_t[:, h * P:(h + 1) * P], po[:], rec[:]
                )

            # output projection for this q tile -> partial y in DRAM
            ats = []
            for hb in range(GH):
                atp = ps_t.tile([P, P], BF16, tag="ptp")
                nc.tensor.transpose(
                    atp[:], attn_t[:, hb * P:(hb + 1) * P], ident[:]
                )
                a = at_pool.tile([P, P], BF16, tag="at")
                nc.vector.tensor_copy(a[:], atp[:])
                ats.append(a)
            for oc in range(TT):
                ps = ps512.tile([P, 512], F32, tag="ps512")
                for hb in range(GH):
                    nc.tensor.matmul(
                        ps[:],
                        lhsT=ats[hb][:],
                        rhs=wo_sb[hb][oc][:],
                        start=(hb == 0),
                        stop=(hb == GH - 1),
                    )
                ysb = ypool.tile([P, 512], F32, tag="y")
                nc.scalar.copy(ysb[:], ps[:])
                nc.sync.dma_start(
                    y_part[qi * P:(qi + 1) * P, oc * 512:(oc + 1) * 512],
                    ysb[:],
                )

        # ---- on-device reduction over the batch group ----
        nc.gpsimd.collective_compute(
            "ReduceScatter", mybir.AluOpType.add,
            replica_groups=GROUPS_BATCH,
            ins=[y_part.opt()], outs=[y_red.opt()],
        )
        # int8-quantize rows through SBUF to shrink the device->host bytes:
        # per-row scale s = rowmax(|y|)/126, emit round(y/s) int8; the f32
        # scale is bit-packed into the last 4 int8 columns of the row.
        y_q8 = dram.tile([GD, D + 4], I8, tag="y_q8", name="y_q8")
        ycvt = ctx.enter_context(tc.tile_pool(name="ycvt", bufs=2))
        for r in range(GD // P):
            tf = ycvt.tile([P, D], F32, tag="ycvt_f")
            nc.sync.dma_start(tf[:], y_red[r * P:(r + 1) * P, :])
            mx = ycvt.tile([P, 1], F32, tag="ymx")
            nc.vector.tensor_reduce(
                mx[:], tf[:], axis=mybir.AxisListType.X,
                op=mybir.AluOpType.max, apply_absolute_value=True,
            )
            sc = ycvt.tile([P, 1], F32, tag="ysc")
            nc.vector.tensor_scalar_mul(sc[:], mx[:], 1.0 / 126.0)
            nc.sync.dma_start(
                y_q8[r * P:(r + 1) * P, D:D + 4], sc[:].bitcast(I8)
            )
            rcp = ycvt.tile([P, 1], F32, tag="yrcp")
            nc.vector.reciprocal(rcp[:], sc[:])
            tq = ycvt.tile([P, D], F32, tag="ycvt_q")
            nc.vector.tensor_scalar_mul(tq[:], tf[:], rcp[:])
            t8 = ycvt.tile([P, D], I8, tag="ycvt8")
            nc.vector.tensor_copy(t8[:], tq[:])
            nc.sync.dma_start(y_q8[r * P:(r + 1) * P, 0:D], t8[:])
        # gather all 8 slices (batch-major) so every core holds full y
        y_gath = dram.tile([B * S, D + 4], I8, tag="y_gath", name="y_gath")
        nc.gpsimd.collective_compute(
            "AllGather", mybir.AluOpType.bypass,
            replica_groups=[[0, 1, 2, 3, 4, 5, 6, 7]],
            ins=[y_q8.opt()], outs=[y_gath.opt()],
        )
        nc.sync.dma_start(y[:], y_gath[:])
    _split_multi_waits(nc)
    return nc


# ---------------- host-side runner ----------------

_NC_CACHE = None
_RUNNER = None
_last_in_maps = None


class _Runner:
    """Replicates concourse.bass_utils.run_bass_kernel_spmd's axon/PJRT
    path, but caches the jitted executable across calls (the library
    rebuilds + reloads it every call), skips the donated zero output
    buffers (this kernel writes every output element), and deletes
    stale device buffers to keep the axon tunnel memory-stable.

    Inputs are taken as a dict of already-concatenated global arrays
    (shape [8 * per_core_rows, ...]) keyed by parameter name."""

    def __init__(self, nc, n_cores):
        import jax
        from jax.experimental.shard_map import shard_map
        from jax.sharding import Mesh, PartitionSpec
        from concourse import bass2jax
        from concourse import mybir as _mybir

        bass2jax.install_neuronx_cc_hook()
        self._jax = jax
        self.n_cores = n_cores
        partition_name = (
            nc.partition_id_tensor.name if nc.partition_id_tensor else None
        )
        in_names, out_names, out_avals = [], [], []
        for alloc in nc.m.functions[0].allocations:
            if not isinstance(alloc, _mybir.MemoryLocationSet):
                continue
            name = alloc.memorylocations[0].name
            if alloc.kind == "ExternalInput":
                if name != partition_name:
                    in_names.append(name)
            elif alloc.kind == "ExternalOutput":
                out_names.append(name)
                out_avals.append(
                    jax.core.ShapedArray(
                        tuple(alloc.tensor_shape), _mybir.dt.np(alloc.dtype)
                    )
                )
        self.in_names = in_names
        self.out_names = out_names
        self.out_avals = out_avals
        in_names_all = list(in_names)
        if partition_name is not None:
            in_names_all.append(partition_name)

        def _body(*args):
            operands = list(args)
            if partition_name is not None:
                operands.append(bass2jax.partition_id_tensor())
            outs = bass2jax._bass_exec_p.bind(
                *operands,
                out_avals=tuple(out_avals),
                in_names=tuple(in_names_all),
                out_names=tuple(out_names),
                lowering_input_output_aliases=(),
                sim_require_finite=True,
                sim_require_nnan=True,
                nc=nc,
            )
            return tuple(outs)

        devices = jax.devices()[:n_cores]
        assert len(devices) == n_cores
        mesh = Mesh(np.asarray(devices), ("core",))
        in_specs = (PartitionSpec("core"),) * len(in_names)
        # every core writes the identical full y (final on-device
        # AllGather), so the output is replicated: the host fetches a
        # single replica instead of 8 shards.
        out_specs = (PartitionSpec(),) * len(out_names)
        self._fn = jax.jit(
            shard_map(
                _body, mesh=mesh, in_specs=in_specs, out_specs=out_specs,
                check_rep=False,
            ),
            keep_unused=True,
        )

    def __call__(self, arrs):
        out_arrs = self._fn(*[arrs[name] for name in self.in_names])
        for o in out_arrs:  # issue all fetches before assembling
            o.copy_to_host_async()
        outs = {
            name: np.asarray(o)
            for name, o in zip(self.out_names, out_arrs)
        }
        for o in out_arrs:  # free remote buffers eagerly
            o.delete()
        return outs


def _get_runner():
    global _NC_CACHE, _RUNNER
    if _RUNNER is None:
        _NC_CACHE = build_bass()
        _RUNNER = _Runner(_NC_CACHE, N_CORES)
    return _RUNNER


def _quant_rows(x, levels=127.0):
    """Per-row absmax int8 quantization of a 2-D array. Returns (i8, s)."""
    s = np.abs(x).max(axis=1) / levels
    s[s == 0] = 1.0
    q = np.rint(x / s[:, None]).astype(np.int8)
    return q, s


def _prep_inputs(inputs):
    """Build the globally-concatenated per-parameter arrays directly."""
    query = np.asarray(inputs["query"], np.float32)
    key = np.asarray(inputs["key"], np.float32)
    value = np.asarray(inputs["value"], np.float32)
    Wq = np.asarray(inputs["Wq"], np.float32)
    bq = np.asarray(inputs["bq"], np.float32)
    Wk = np.asarray(inputs["Wk"], np.float32)
    bk = np.asarray(inputs["bk"], np.float32)
    Wv = np.asarray(inputs["Wv"], np.float32)
    Wo = np.asarray(inputs["Wo"], np.float32)

    # weights: int8 with per-contraction-row (= per input-column) scales
    # over the full output dim, so scales are head-group independent.
    w_i8t, w_s = {}, {}
    for nm, W in (("q", Wq), ("k", Wk), ("v", Wv), ("o", Wo)):
        s = np.abs(W).max(axis=0) / 127.0
        s[s == 0] = 1.0
        w_i8t[nm] = np.rint(W.T / s[:, None]).astype(np.int8)  # [d_in, d_out]
        w_s[nm] = s

    xw = np.zeros((N_CORES * XW_ROWS, GD), np.int8)

    for c in range(N_CORES):
        b, g, hb = c // 4, c % 4, c // 4
        gsl = slice(GD * g, GD * (g + 1))
        r0 = c * XW_ROWS
        scl = np.zeros((P, SCL_C), np.float32)
        for j, (x, wn) in enumerate(((query, "q"), (key, "k"), (value, "v"))):
            blk = x[b, gsl, :].T              # [D, 512] feature rows
            qi8, s = _quant_rows(blk)
            xw[r0 + j * D:r0 + (j + 1) * D] = qi8
            # combined scale = act scale * matching weight scale, laid out
            # [128, 16] with partition p <-> d = kc*128+p
            scl[:, j * KC_COLS:(j + 1) * KC_COLS] = (
                (s * w_s[wn]).reshape(D // P, P).T
            )
        # wo scales: full 512 rows of this group as 4 chunks of 128
        scl[:, 48:52] = w_s["o"][gsl].reshape(GH, P).T
        scl[:, 52:56] = (bq[gsl] * SCALE).reshape(GH, P).T
        scl[:, 56:60] = bk[gsl].reshape(GH, P).T
        xw[r0 + R_ACTS:r0 + R_ACTS + P, 0:4 * SCL_C] = scl.view(np.int8)
        wsl = slice((D // 2) * hb, (D // 2) * (hb + 1))
        w0 = r0 + R_W
        xw[w0 + 0 * (D // 2):w0 + 1 * (D // 2)] = w_i8t["q"][wsl, gsl]
        xw[w0 + 1 * (D // 2):w0 + 2 * (D // 2)] = w_i8t["k"][wsl, gsl]
        xw[w0 + 2 * (D // 2):w0 + 3 * (D // 2)] = w_i8t["v"][wsl, gsl]
        # wo half: 8 partition-friendly [128, 512] tiles (lc, oc)
        w1 = w0 + 3 * (D // 2)
        for lc in range(2):
            for oc in range(4):
                a0 = GD * g + (GD // 2) * hb + lc * P
                xw[w1 + (lc * 4 + oc) * P:w1 + (lc * 4 + oc + 1) * P] = (
                    w_i8t["o"][a0:a0 + P, oc * 512:(oc + 1) * 512]
                )

    return {"xw": xw}


KC_COLS = D // P  # 16 scale columns per tensor in scl


def _gather(results, bo_eff):
    yi8 = results["y"]                       # [B*S, D+4] batch-major
    sc = yi8[:, D:D + 4].copy().view(np.float32)
    out = yi8[:, :D].astype(np.float32) * sc + bo_eff[None, :]
    return out.reshape(B, S, D)


def kernel(**inputs):
    global _last_in_maps
    bo = np.asarray(inputs["bo"], np.float32)
    bv = np.asarray(inputs["bv"], np.float32)
    Wo = np.asarray(inputs["Wo"], np.float32)
    bo_eff = bo + bv @ Wo.T   # V bias folds through softmax-normalized P
    arrs = _prep_inputs(inputs)
    _last_in_maps = arrs
    runner = _get_runner()
    results = runner(arrs)
    return _gather(results, bo_eff)


# revision 15
# speedup vs baseline: 1.0613x; 1.0613x over previous
"""Trainium2 Bass kernel for KV-cached (causal) multi-head attention.

Full module: y = softmax(mask(QK^T/sqrt(hd))) V  -> out_proj, with
Q/K/V = linear projections of query/key/value inputs.

Shapes (hardcoded): B=2, S=2048, D=2048, H=16 heads, hd=128.

Sharding (8 NeuronCores): core c handles batch b=c//4 and head group
g=c%4 (4 heads = 512 dims).  The host<->device tunnel is the wall-time
bottleneck (~50-80 MB/s shared pipe), so all bulk traffic is int8:

  - activations: core c receives the c%4-th 512-row slice of its
    batch's query/key/value, transposed to [D, 512] and int8-quantized
    per feature row (scale = absmax/127 over the 512 seq positions).
    An AllGather over the batch group {4b..4b+3} rebuilds the full
    [D, S] transposed activations as 4 column blocks.
  - weights: int8 with per-contraction-row scales taken over the FULL
    output row (so the scale is head-group independent); cores c and
    c+4 need identical TP weight slices, so each receives half and an
    AllGather over pairs {c, c+4} rebuilds them.  Because both the
    activation scale and the weight scale are per-contraction-dim, the
    host pre-multiplies them into a single combined scale applied to
    the activation tiles on device; the int8 weights are then used
    EXACTLY (integers <=127 are exact in fp16/bf16).
  - the V bias is folded into the output bias on the host
    (softmax rows sum to 1, so P(V + bv) = PV + bv after
    normalization => bo' = bo + bv @ Wo.T), removing it from the
    device entirely.
  - output: partial out-projections are summed on device with a
    ReduceScatter over the batch group; each core returns only its
    512-row slice of y, int8-quantized with per-row f32 scales that
    are bit-packed into 4 extra int8 columns (single output tensor).

On-device layout (fp16 matmuls on the Q/K path for extra mantissa,
bf16 elsewhere, fp32 PSUM accumulation):
  - Q^T, K^T computed as [dq, S] (head dim on partitions) so that
    scores = Q^T.T @ K^T needs no on-device transposes
  - V computed as [S, dv]
  - softmax per q-row (partition) along free kv axis; exp on ScalarE
    with fused per-chunk row-sums (accum_out); causal handled by
    skipping kv blocks beyond the diagonal + one additive mask tile
    on the diagonal 128x128 block
  - P^T for the PV matmul via PE-mode transposes of 128x128 blocks
  - attention output [q, hd] re-transposed per 128-block to feed the
    output projection as lhsT
"""

import sys

for _p in ("/opt/trn_rl_repo",):
    if _p not in sys.path:
        sys.path.insert(0, _p)

from contextlib import ExitStack

import numpy as np

import concourse.bass as bass
import concourse.mybir as mybir
import concourse.tile as tile
from concourse.vector_clock import ScopedClock
from concourse.masks import make_causal_mask, make_identity

BF16 = mybir.dt.bfloat16
FP16 = mybir.dt.float16
F32 = mybir.dt.float32
I8 = mybir.dt.int8

B, S, D = 2, 2048, 2048
NH, HD = 16, 128          # total heads, head dim
GH = 4                    # heads per core
GD = GH * HD              # 512 dims per core
P = 128
SCALE = 1.0 / np.sqrt(HD)
N_CORES = 8

GROUPS_BATCH = [[0, 1, 2, 3], [4, 5, 6, 7]]   # share one batch's acts
GROUPS_PAIR = [[0, 4], [1, 5], [2, 6], [3, 7]]  # share TP weight slices

# Single packed int8 input, [XW_ROWS, 512] rows per core:
#   rows 0..6143     : qT/kT/vT activation slices (2048 rows each)
#   rows 6144..6271  : f32 scale/bias block bit-packed into int8 (240 of
#                      512 bytes per row used); AllGathered with the acts
#   rows 6272..9343  : wq/wk/wv int8 half-slices (1024 rows each)
#   rows 9344..10367 : wo int8 half, packed as 8 [128, 512] tiles
#                      (lc, oc) so gathered tiles stay partition-friendly
# scl columns: 0..47 combined act*weight scales (j*16+kc), 48..51 wo row
# scales (full 512 of this group as 4 chunks), 52..55 bq*SCALE, 56..59 bk
R_ACTS = 3 * D                   # 6144
BATCH_BLK = R_ACTS + P           # 6272 rows gathered over the batch group
R_W = BATCH_BLK                  # weight half rows start here
XW_ROWS = BATCH_BLK + 3 * (D // 2) + (D // 2)   # 10368
SCL_C = 60


def _drain_and_barrier_split(self, tick_clock, wait_clock):
    # The walrus build in this container rejects a Drain carrying more
    # than one sync wait ("Too many sync wait commands").  Semantically
    # equivalent: chain one drain per wait on the sync engine.
    nc = self.nc
    drain_inst = nc.sync.drain()
    wait_clock.add_sem_waits(
        drain_inst.ins, ScopedClock({None: tick_clock.global_clock})
    )
    si = drain_inst.ins.sync_info
    waits = list(si.on_wait)
    if len(waits) > 1:
        drain_inst.ins.sync_info = mybir.SyncInfo(
            on_wait=[waits[0]], on_update=list(si.on_update)
        )
        for w in waits[1:]:
            d = nc.sync.drain()
            d.ins.sync_info = mybir.SyncInfo(on_wait=[w], on_update=[])
    nc.all_engine_barrier()
    assert self.sems is not None
    popped = nc._tile_sem_poison_stack.pop()
    assert popped is self._sem_poison
    nc.clear_and_free_semaphores(list(self.sems.allocated().values()))
    nc.all_engine_barrier()


tile.TileContext._drain_and_barrier = _drain_and_barrier_split


def _split_multi_waits(nc, max_waits=1):
    """This container's walrus rejects instructions carrying more than one
    sync wait.  Hoist extra waits onto same-engine NoOps placed just before
    the instruction (waits execute in engine program order, so this is
    semantically identical)."""
    uid = [0]
    for fn in nc.m.functions:
        for bb in fn.blocks:
            insts = bb.instructions
            new = []
            changed = False
            for inst in insts:
                si = getattr(inst, "sync_info", None)
                waits = list(si.on_wait) if si is not None else []
                if len(waits) > max_waits:
                    changed = True
                    n_keep = max_waits
                    for w in waits[:-n_keep]:
                        nop = mybir.InstNoOp(
                            name=f"WSPLIT-{uid[0]}", ins=[], outs=[]
                        )
                        uid[0] += 1
                        nop.engine = inst.engine
                        nop.sync_info = mybir.SyncInfo(
                            on_wait=[w], on_update=[]
                        )
                        new.append(nop)
                    inst.sync_info = mybir.SyncInfo(
                        on_wait=waits[-n_keep:], on_update=list(si.on_update)
                    )
                new.append(inst)
            if changed:
                bb.instructions = new
    return nc


def build_bass():
    nc = bass.Bass(num_devices=N_CORES)
    xw_in = nc.declare_dram_parameter("xw", [XW_ROWS, GD], I8, isOutput=False)
    # full batch-major int8 y (+bit-packed f32 row scales), identical on
    # every core via a final AllGather so the host fetches ONE replica
    y = nc.declare_dram_parameter("y", [B * S, D + 4], I8, isOutput=True)

    KC = D // P               # 16 contraction chunks of 128
    TT = S // 512             # 4 t-tiles of 512
    QI = S // P               # 16 q tiles of 128

    with tile.TileContext(nc) as tc, ExitStack() as ctx:
        # ---- DRAM staging + collectives ----
        dram = ctx.enter_context(tc.tile_pool(name="dram", bufs=1, space="DRAM"))
        xw_loc = dram.tile([XW_ROWS, GD], I8, tag="xw_loc")
        xs_g = dram.tile([4 * BATCH_BLK, GD], I8, tag="xs_g")
        w_g = dram.tile([2 * (XW_ROWS - R_W), GD], I8, tag="w_g", name="w_g")
        y_part = dram.tile([S, D], F32, tag="y_part")
        y_red = dram.tile([GD, D], F32, tag="y_red")

        nc.sync.dma_start(xw_loc[:], xw_in[:])
        nc.gpsimd.collective_compute(
            "AllGather", mybir.AluOpType.bypass,
            replica_groups=GROUPS_BATCH,
            ins=[xw_loc[0:BATCH_BLK, :].opt()], outs=[xs_g.opt()],
        )
        nc.gpsimd.collective_compute(
            "AllGather", mybir.AluOpType.bypass,
            replica_groups=GROUPS_PAIR,
            ins=[xw_loc[R_W:XW_ROWS, :].opt()], outs=[w_g.opt()],
        )
        # weight-half h (0/1) of the pair lives at w_g rows h*4096..
        WH = XW_ROWS - R_W        # 4096 rows per half

        const = ctx.enter_context(tc.tile_pool(name="const", bufs=1))
        maskt = const.tile([P, P], F32)
        make_causal_mask(nc, maskt, mask_val=-1e9)
        ident = const.tile([P, P], BF16)
        make_identity(nc, ident)
        scl_sb = const.tile([P, SCL_C], F32)
        nc.sync.dma_start(
            scl_sb[:], xw_loc[R_ACTS:R_ACTS + P, 0:4 * SCL_C].bitcast(F32)
        )
        sclg_sb = []
        for tt in range(TT):
            t = const.tile([P, SCL_C], F32, tag=f"sclg{tt}")
            r = tt * BATCH_BLK + R_ACTS
            nc.sync.dma_start(t[:], xs_g[r:r + P, 0:4 * SCL_C].bitcast(F32))
            sclg_sb.append(t)
        bq_sb = scl_sb[:, 52:52 + GH]
        bk_sb = scl_sb[:, 56:56 + GH]

        # resident weights: int8 converted to fp16 (q/k) / bf16 (v, wo);
        # integer values are exact in 16-bit floats.
        wpool = ctx.enter_context(tc.tile_pool(name="weights", bufs=1))
        wst = ctx.enter_context(tc.tile_pool(name="wst", bufs=6))
        wq_sb, wk_sb, wv_sb = [], [], []
        for i, (name, lst, dt_) in enumerate((
            ("wq", wq_sb, FP16),
            ("wk", wk_sb, FP16),
            ("wv", wv_sb, BF16),
        )):
            for kc in range(KC):
                st = wst.tile([P, GD], I8, tag="wst")
                r = (kc // 8) * WH + i * (D // 2) + (kc % 8) * P
                nc.sync.dma_start(st[:], w_g[r:r + P, :])
                t = wpool.tile([P, GD], dt_, name=f"{name}{kc}", tag=f"{name}{kc}")
                nc.vector.tensor_copy(t[:], st[:])
                lst.append(t)
        # wo: 16 tiles [128, 512] indexed [hb][oc]; gathered tile (hb, oc)
        # sits at w_g row (hb//2)*WH + 3072 + ((hb%2)*4 + oc)*128
        wo_sb = []
        for hb in range(GH):
            row = []
            for oc in range(TT):
                st = wst.tile([P, GD], I8, tag="wst")
                r = (hb // 2) * WH + 3 * (D // 2) + ((hb % 2) * 4 + oc) * P
                nc.sync.dma_start(st[:], w_g[r:r + P, :])
                t = wpool.tile([P, GD], BF16, name=f"woc{hb}_{oc}",
                               tag=f"wo{hb}_{oc}")
                nc.scalar.activation(
                    t[:], st[:], mybir.ActivationFunctionType.Identity,
                    scale=scl_sb[:, 48 + hb:49 + hb],
                )
                row.append(t)
            wo_sb.append(row)

        # persistent activations
        act = ctx.enter_context(tc.tile_pool(name="acts", bufs=1))
        qT_sb = [act.tile([P, S], FP16, name=f"qT{h}", tag=f"qT{h}") for h in range(GH)]
        kT_sb = [act.tile([P, S], FP16, name=f"kT{h}", tag=f"kT{h}") for h in range(GH)]
        v_sb = [act.tile([P, GD], BF16, name=f"v{i}", tag=f"v{i}") for i in range(QI)]

        ctxA = ExitStack()
        xin = ctxA.enter_context(tc.tile_pool(name="xin", bufs=24))
        xdq = ctxA.enter_context(tc.tile_pool(name="xdq", bufs=24))
        ps512 = ctx.enter_context(
            tc.tile_pool(name="ps512", bufs=4, space="PSUM")
        )

        # xs_g row offset for (column-block tt, tensor j, contraction chunk kc)
        def _xrow(tt, j, kc):
            return tt * BATCH_BLK + j * D + kc * P

        # ---- Q^T / K^T projections: out [dq=512, S] in fp16 ----
        for j, (w_sb, out_tiles, b_tile, scale) in enumerate((
            (wq_sb, qT_sb, bq_sb, SCALE),
            (wk_sb, kT_sb, bk_sb, 1.0),
        )):
            for tt in range(TT):
                xch = []
                for kc in range(KC):
                    ti = xin.tile([P, 512], I8, tag="xin")
                    r = _xrow(tt, j, kc)
                    nc.sync.dma_start(ti[:], xs_g[r:r + P, :])
                    td = xdq.tile([P, 512], FP16, tag="xdq")
                    c = j * KC + kc
                    nc.scalar.activation(
                        td[:], ti[:], mybir.ActivationFunctionType.Identity,
                        scale=sclg_sb[tt][:, c:c + 1],
                    )
                    xch.append(td)
                for dt in range(GH):
                    ps = ps512.tile([P, 512], F32, tag="ps512")
                    for kc in range(KC):
                        nc.tensor.matmul(
                            ps[:],
                            lhsT=w_sb[kc][:, dt * P:(dt + 1) * P],
                            rhs=xch[kc][:],
                            start=(kc == 0),
                            stop=(kc == KC - 1),
                        )
                    # evict: out = (psum + b) * scale, bias pre-scaled on host
                    nc.scalar.activation(
                        out_tiles[dt][:, tt * 512:(tt + 1) * 512],
                        ps[:],
                        mybir.ActivationFunctionType.Identity,
                        bias=b_tile[:, dt:dt + 1],
                        scale=scale,
                    )

        # ---- V projection: out [S, dv=512] in bf16, no bias (folded) ----
        for ttg in range(TT):
            xch = []
            for kc in range(KC):
                ti = xin.tile([P, 512], I8, tag="xin")
                r = _xrow(ttg, 2, kc)
                nc.sync.dma_start(ti[:], xs_g[r:r + P, :])
                td = xdq.tile([P, 512], BF16, tag="xdq")
                c = 2 * KC + kc
                nc.scalar.activation(
                    td[:], ti[:], mybir.ActivationFunctionType.Identity,
                    scale=sclg_sb[ttg][:, c:c + 1],
                )
                xch.append(td)
            for sub in range(4):
                ps = ps512.tile([P, 512], F32, tag="ps512")
                for kc in range(KC):
                    nc.tensor.matmul(
                        ps[:],
                        lhsT=xch[kc][:, sub * P:(sub + 1) * P],
                        rhs=wv_sb[kc][:],
                        start=(kc == 0),
                        stop=(kc == KC - 1),
                    )
                nc.vector.tensor_copy(v_sb[ttg * 4 + sub][:], ps[:])

        ctxA.close()

        # ---- attention + output projection, per q tile ----
        ppool = ctx.enter_context(tc.tile_pool(name="p", bufs=2))
        spool = ctx.enter_context(tc.tile_pool(name="sums", bufs=8))
        ps_t = ctx.enter_context(tc.tile_pool(name="ps_t", bufs=2, space="PSUM"))
        ps_o = ctx.enter_context(tc.tile_pool(name="ps_o", bufs=2, space="PSUM"))
        ptp_pool = ctx.enter_context(tc.tile_pool(name="pt", bufs=3))
        at_pool = ctx.enter_context(tc.tile_pool(name="at", bufs=5))
        attn_pool = ctx.enter_context(tc.tile_pool(name="attn", bufs=2))
        ypool = ctx.enter_context(tc.tile_pool(name="ysb", bufs=3))

        for qi in range(QI):
            kv_len = (qi + 1) * P
            nchunks = (kv_len + 511) // 512
            attn_t = attn_pool.tile([P, GD], BF16, tag="attn")
            for h in range(GH):
                p_t = ppool.tile([P, S], BF16, tag="p")
                sums = spool.tile([P, 4], F32, tag="sums")
                for c in range(nchunks):
                    n = min(512, kv_len - c * 512)
                    ps = ps512.tile([P, 512], F32, tag="ps512")
                    nc.tensor.matmul(
                        ps[:, :n],
                        lhsT=qT_sb[h][:, qi * P:(qi + 1) * P],
                        rhs=kT_sb[h][:, c * 512:c * 512 + n],
                        start=True,
                        stop=True,
                    )
                    if c == nchunks - 1:
                        nc.vector.tensor_add(
                            ps[:, n - P:n], ps[:, n - P:n], maskt[:]
                        )
                    nc.scalar.activation(
                        p_t[:, c * 512:c * 512 + n],
                        ps[:, :n],
                        mybir.ActivationFunctionType.Exp,
                        accum_out=sums[:, c:c + 1],
                    )
                tot = spool.tile([P, 1], F32, tag="tot")
                nc.vector.reduce_sum(
                    tot[:], sums[:, :nchunks], axis=mybir.AxisListType.X
                )
                rec = spool.tile([P, 1], F32, tag="rec")
                nc.vector.reciprocal(rec[:], tot[:])

                po = ps_o.tile([P, P], F32)
                pts = {}

                def _pv_transpose(kb):
                    ptp = ps_t.tile([P, P], BF16, tag="ptp")
                    nc.tensor.transpose(
                        ptp[:], p_t[:, kb * P:(kb + 1) * P], ident[:]
                    )
                    s = ptp_pool.tile([P, P], BF16, tag="pt")
                    nc.vector.tensor_copy(s[:], ptp[:])
                    pts[kb] = s

                # pipeline transposes one block ahead of the PV matmuls so
                # the PE never waits on the DVE copy of the current block
                _pv_transpose(0)
                for kb in range(qi + 1):
                    if kb + 1 <= qi:
                        _pv_transpose(kb + 1)
                    nc.tensor.matmul(
                        po[:],
                        lhsT=pts.pop(kb)[:],
                        rhs=v_sb[kb][:, h * P:(h + 1) * P],
                        start=(kb == 0),
                        stop=(kb == qi),
                    )
                nc.vector.tensor_scalar_mul(
                    attn_t[:, h * P:(h + 1) * P], po[:], rec[:]
                )

            # output projection for this q tile -> partial y in DRAM
            ats = []
            for hb in range(GH):
                atp = ps_t.tile([P, P], BF16, tag="ptp")
                nc.tensor.transpose(
                    atp[:], attn_t[:, hb * P:(hb + 1) * P], ident[:]
                )
                a = at_pool.tile([P, P], BF16, tag="at")
                nc.vector.tensor_copy(a[:], atp[:])
                ats.append(a)
            for oc in range(TT):
                ps = ps512.tile([P, 512], F32, tag="ps512")
                for hb in range(GH):
                    nc.tensor.matmul(
                        ps[:],
                        lhsT=ats[hb][:],
                        rhs=wo_sb[hb][oc][:],
                        start=(hb == 0),
                        stop=(hb == GH - 1),
                    )
                ysb = ypool.tile([P, 512], F32, tag="y")
                nc.scalar.copy(ysb[:], ps[:])
                nc.sync.dma_start(
                    y_part[qi * P:(qi + 1) * P, oc * 512:(oc + 1) * 512],
                    ysb[:],
                )

        # ---- on-device reduction over the batch group ----
        nc.gpsimd.collective_compute(
            "ReduceScatter", mybir.AluOpType.add,
            replica_groups=GROUPS_BATCH,
            ins=[y_part.opt()], outs=[y_red.opt()],
        )
        # int8-quantize rows through SBUF to shrink the device->host bytes:
        # per-row scale s = rowmax(|y|)/126, emit round(y/s) int8; the f32
        # scale is bit-packed into the last 4 int8 columns of the row.
        y_q8 = dram.tile([GD, D + 4], I8, tag="y_q8", name="y_q8")
        ycvt = ctx.enter_context(tc.tile_pool(name="ycvt", bufs=2))
        for r in range(GD // P):
            tf = ycvt.tile([P, D], F32, tag="ycvt_f")
            nc.sync.dma_start(tf[:], y_red[r * P:(r + 1) * P, :])
            mx = ycvt.tile([P, 1], F32, tag="ymx")
            nc.vector.tensor_reduce(
                mx[:], tf[:], axis=mybir.AxisListType.X,
                op=mybir.AluOpType.max, apply_absolute_value=True,
            )
            sc = ycvt.tile([P, 1], F32, tag="ysc")
            nc.vector.tensor_scalar_mul(sc[:], mx[:], 1.0 / 126.0)
            nc.sync.dma_start(
                y_q8[r * P:(r + 1) * P, D:D + 4], sc[:].bitcast(I8)
            )
            rcp = ycvt.tile([P, 1], F32, tag="yrcp")
            nc.vector.reciprocal(rcp[:], sc[:])
            tq = ycvt.tile([P, D], F32, tag="ycvt_q")
            nc.vector.tensor_scalar_mul(tq[:], tf[:], rcp[:])
            t8 = ycvt.tile([P, D], I8, tag="ycvt8")
            nc.vector.tensor_copy(t8[:], tq[:])
            nc.sync.dma_start(y_q8[r * P:(r + 1) * P, 0:D], t8[:])
        # gather all 8 slices (batch-major) so every core holds full y
        y_gath = dram.tile([B * S, D + 4], I8, tag="y_gath", name="y_gath")
        nc.gpsimd.collective_compute(
            "AllGather", mybir.AluOpType.bypass,
            replica_groups=[[0, 1, 2, 3, 4, 5, 6, 7]],
            ins=[y_q8.opt()], outs=[y_gath.opt()],
        )
        nc.sync.dma_start(y[:], y_gath[:])
    _split_multi_waits(nc)
    return nc


# ---------------- host-side runner ----------------

_NC_CACHE = None
_RUNNER = None
_last_in_maps = None


class _Runner:
    """Replicates concourse.bass_utils.run_bass_kernel_spmd's axon/PJRT
    path, but caches the jitted executable across calls (the library
    rebuilds + reloads it every call), skips the donated zero output
    buffers (this kernel writes every output element), and deletes
    stale device buffers to keep the axon tunnel memory-stable.

    Inputs are taken as a dict of already-concatenated global arrays
    (shape [8 * per_core_rows, ...]) keyed by parameter name."""

    def __init__(self, nc, n_cores):
        import jax
        from jax.experimental.shard_map import shard_map
        from jax.sharding import Mesh, PartitionSpec
        from concourse import bass2jax
        from concourse import mybir as _mybir

        bass2jax.install_neuronx_cc_hook()
        self._jax = jax
        self.n_cores = n_cores
        partition_name = (
            nc.partition_id_tensor.name if nc.partition_id_tensor else None
        )
        in_names, out_names, out_avals = [], [], []
        for alloc in nc.m.functions[0].allocations:
            if not isinstance(alloc, _mybir.MemoryLocationSet):
                continue
            name = alloc.memorylocations[0].name
            if alloc.kind == "ExternalInput":
                if name != partition_name:
                    in_names.append(name)
            elif alloc.kind == "ExternalOutput":
                out_names.append(name)
                out_avals.append(
                    jax.core.ShapedArray(
                        tuple(alloc.tensor_shape), _mybir.dt.np(alloc.dtype)
                    )
                )
        self.in_names = in_names
        self.out_names = out_names
        self.out_avals = out_avals
        in_names_all = list(in_names)
        if partition_name is not None:
            in_names_all.append(partition_name)

        def _body(*args):
            operands = list(args)
            if partition_name is not None:
                operands.append(bass2jax.partition_id_tensor())
            outs = bass2jax._bass_exec_p.bind(
                *operands,
                out_avals=tuple(out_avals),
                in_names=tuple(in_names_all),
                out_names=tuple(out_names),
                lowering_input_output_aliases=(),
                sim_require_finite=True,
                sim_require_nnan=True,
                nc=nc,
            )
            return tuple(outs)

        devices = jax.devices()[:n_cores]
        assert len(devices) == n_cores
        mesh = Mesh(np.asarray(devices), ("core",))
        in_specs = (PartitionSpec("core"),) * len(in_names)
        # every core writes the identical full y (final on-device
        # AllGather), so the output is replicated: the host fetches a
        # single replica instead of 8 shards.
        out_specs = (PartitionSpec(),) * len(out_names)
        self._fn = jax.jit(
            shard_map(
                _body, mesh=mesh, in_specs=in_specs, out_specs=out_specs,
                check_rep=False,
            ),
            keep_unused=True,
        )

    def __call__(self, arrs):
        out_arrs = self._fn(*[arrs[name] for name in self.in_names])
        outs = {
            name: np.asarray(o)
            for name, o in zip(self.out_names, out_arrs)
        }
        for o in out_arrs:  # free remote buffers eagerly
            o.delete()
        return outs


def _get_runner():
    global _NC_CACHE, _RUNNER
    if _RUNNER is None:
        _NC_CACHE = build_bass()
        _RUNNER = _Runner(_NC_CACHE, N_CORES)
    return _RUNNER


def _quant_rows(x, levels=127.0):
    """Per-row absmax int8 quantization of a 2-D array. Returns (i8, s)."""
    s = np.abs(x).max(axis=1) / levels
    s[s == 0] = 1.0
    q = np.rint(x / s[:, None]).astype(np.int8)
    return q, s


def _prep_inputs(inputs):
    """Build the globally-concatenated per-parameter arrays directly."""
    query = np.asarray(inputs["query"], np.float32)
    key = np.asarray(inputs["key"], np.float32)
    value = np.asarray(inputs["value"], np.float32)
    Wq = np.asarray(inputs["Wq"], np.float32)
    bq = np.asarray(inputs["bq"], np.float32)
    Wk = np.asarray(inputs["Wk"], np.float32)
    bk = np.asarray(inputs["bk"], np.float32)
    Wv = np.asarray(inputs["Wv"], np.float32)
    Wo = np.asarray(inputs["Wo"], np.float32)

    # weights: int8 with per-contraction-row (= per input-column) scales
    # over the full output dim, so scales are head-group independent.
    w_i8t, w_s = {}, {}
    for nm, W in (("q", Wq), ("k", Wk), ("v", Wv), ("o", Wo)):
        s = np.abs(W).max(axis=0) / 127.0
        s[s == 0] = 1.0
        w_i8t[nm] = np.rint(W.T / s[:, None]).astype(np.int8)  # [d_in, d_out]
        w_s[nm] = s

    xw = np.zeros((N_CORES * XW_ROWS, GD), np.int8)

    for c in range(N_CORES):
        b, g, hb = c // 4, c % 4, c // 4
        gsl = slice(GD * g, GD * (g + 1))
        r0 = c * XW_ROWS
        scl = np.zeros((P, SCL_C), np.float32)
        for j, (x, wn) in enumerate(((query, "q"), (key, "k"), (value, "v"))):
            blk = x[b, gsl, :].T              # [D, 512] feature rows
            qi8, s = _quant_rows(blk)
            xw[r0 + j * D:r0 + (j + 1) * D] = qi8
            # combined scale = act scale * matching weight scale, laid out
            # [128, 16] with partition p <-> d = kc*128+p
            scl[:, j * KC_COLS:(j + 1) * KC_COLS] = (
                (s * w_s[wn]).reshape(D // P, P).T
            )
        # wo scales: full 512 rows of this group as 4 chunks of 128
        scl[:, 48:52] = w_s["o"][gsl].reshape(GH, P).T
        scl[:, 52:56] = (bq[gsl] * SCALE).reshape(GH, P).T
        scl[:, 56:60] = bk[gsl].reshape(GH, P).T
        xw[r0 + R_ACTS:r0 + R_ACTS + P, 0:4 * SCL_C] = scl.view(np.int8)
        wsl = slice((D // 2) * hb, (D // 2) * (hb + 1))
        w0 = r0 + R_W
        xw[w0 + 0 * (D // 2):w0 + 1 * (D // 2)] = w_i8t["q"][wsl, gsl]
        xw[w0 + 1 * (D // 2):w0 + 2 * (D // 2)] = w_i8t["k"][wsl, gsl]
        xw[w0 + 2 * (D // 2):w0 + 3 * (D // 2)] = w_i8t["v"][wsl, gsl]
        # wo half: 8 partition-friendly [128, 512] tiles (lc, oc)
        w1 = w0 + 3 * (D // 2)
        for lc in range(2):
            for oc in range(4):
                a0 = GD * g + (GD // 2) * hb + lc * P
                xw[w1 + (lc * 4 + oc) * P:w1 + (lc * 4 + oc + 1) * P] = (
                    w_i8t["o"][a0:a0 + P, oc * 512:(oc + 1) * 512]
                )

    return {"xw": xw}


KC_COLS = D // P  # 16 scale columns per tensor in scl


def _gather(results, bo_eff):
    yi8 = results["y"]                       # [B*S, D+4] batch-major
    sc = yi8[:, D:D + 4].copy().view(np.float32)
    out = yi8[:, :D].astype(np.float32) * sc + bo_eff[None, :]
    return out.reshape(B, S, D)


def kernel(**inputs):
    global _last_in_maps
    bo = np.asarray(inputs["bo"], np.float32)
    bv = np.asarray(inputs["bv"], np.float32)
    Wo = np.asarray(inputs["Wo"], np.float32)
    bo_eff = bo + bv @ Wo.T   # V bias folds through softmax-normalized P
    arrs = _prep_inputs(inputs)
    _last_in_maps = arrs
    runner = _get_runner()
    results = runner(arrs)
    return _gather(results, bo_eff)


# revision 22
# speedup vs baseline: 1.0636x; 1.0021x over previous
"""Trainium2 Bass kernel for KV-cached (causal) multi-head attention.

Full module: y = softmax(mask(QK^T/sqrt(hd))) V  -> out_proj, with
Q/K/V = linear projections of query/key/value inputs.

Shapes (hardcoded): B=2, S=2048, D=2048, H=16 heads, hd=128.

Sharding (8 NeuronCores): core c handles batch b=c//4 and head group
g=c%4 (4 heads = 512 dims).  The host<->device tunnel is the wall-time
bottleneck (~50-80 MB/s shared pipe), so all bulk traffic is int8:

  - activations: core c receives the c%4-th 512-row slice of its
    batch's query/key/value, transposed to [D, 512] and int8-quantized
    per feature row (scale = absmax/127 over the 512 seq positions).
    An AllGather over the batch group {4b..4b+3} rebuilds the full
    [D, S] transposed activations as 4 column blocks.
  - weights: int8 with per-contraction-row scales taken over the FULL
    output row (so the scale is head-group independent); cores c and
    c+4 need identical TP weight slices, so each receives half and an
    AllGather over pairs {c, c+4} rebuilds them.  Because both the
    activation scale and the weight scale are per-contraction-dim, the
    host pre-multiplies them into a single combined scale applied to
    the activation tiles on device; the int8 weights are then used
    EXACTLY (integers <=127 are exact in fp16/bf16).
  - the V bias is folded into the output bias on the host
    (softmax rows sum to 1, so P(V + bv) = PV + bv after
    normalization => bo' = bo + bv @ Wo.T), removing it from the
    device entirely.
  - output: partial out-projections are summed on device with a
    ReduceScatter over the batch group; each core returns only its
    512-row slice of y, int8-quantized with per-row f32 scales that
    are bit-packed into 4 extra int8 columns (single output tensor).

On-device layout (fp16 matmuls on the Q/K path for extra mantissa,
bf16 elsewhere, fp32 PSUM accumulation):
  - Q^T, K^T computed as [dq, S] (head dim on partitions) so that
    scores = Q^T.T @ K^T needs no on-device transposes
  - V computed as [S, dv]
  - softmax per q-row (partition) along free kv axis; exp on ScalarE
    with fused per-chunk row-sums (accum_out); causal handled by
    skipping kv blocks beyond the diagonal + one additive mask tile
    on the diagonal 128x128 block
  - P^T for the PV matmul via PE-mode transposes of 128x128 blocks
  - attention output [q, hd] re-transposed per 128-block to feed the
    output projection as lhsT
"""

import sys

for _p in ("/opt/trn_rl_repo",):
    if _p not in sys.path:
        sys.path.insert(0, _p)

from contextlib import ExitStack

import numpy as np

import concourse.bass as bass
import concourse.mybir as mybir
import concourse.tile as tile
from concourse.vector_clock import ScopedClock
from concourse.masks import make_causal_mask, make_identity

BF16 = mybir.dt.bfloat16
FP16 = mybir.dt.float16
F32 = mybir.dt.float32
I8 = mybir.dt.int8

B, S, D = 2, 2048, 2048
NH, HD = 16, 128          # total heads, head dim
GH = 4                    # heads per core
GD = GH * HD              # 512 dims per core
P = 128
SCALE = 1.0 / np.sqrt(HD)
N_CORES = 8

GROUPS_BATCH = [[0, 1, 2, 3], [4, 5, 6, 7]]   # share one batch's acts
GROUPS_PAIR = [[0, 4], [1, 5], [2, 6], [3, 7]]  # share TP weight slices

# Single packed int8 input, [XW_ROWS, 512] rows per core:
#   rows 0..6143     : qT/kT/vT activation slices (2048 rows each)
#   rows 6144..6271  : f32 scale/bias block bit-packed into int8 (240 of
#                      512 bytes per row used); AllGathered with the acts
#   rows 6272..9343  : wq/wk/wv int8 half-slices (1024 rows each)
#   rows 9344..10367 : wo int8 half, packed as 8 [128, 512] tiles
#                      (lc, oc) so gathered tiles stay partition-friendly
# scl columns: 0..47 combined act*weight scales (j*16+kc), 48..51 wo row
# scales (full 512 of this group as 4 chunks), 52..55 bq*SCALE, 56..59 bk
R_ACTS = 3 * D                   # 6144
BATCH_BLK = R_ACTS + P           # 6272 rows gathered over the batch group
R_W = BATCH_BLK                  # weight half rows start here
XW_ROWS = BATCH_BLK + 3 * (D // 2) + (D // 2)   # 10368
SCL_C = 60
# y rows travel as 7-bit values bit-packed 8-into-7 bytes: byte i of each
# group carries v_i's low 7 bits, and bit 7 of bytes 0..6 carries bit i
# of v_7.  2048 cols -> 1792 packed bytes + 4 bytes of f32 row scale.
YP = (D // 8) * 7                # 1792


def _drain_and_barrier_split(self, tick_clock, wait_clock):
    # The walrus build in this container rejects a Drain carrying more
    # than one sync wait ("Too many sync wait commands").  Semantically
    # equivalent: chain one drain per wait on the sync engine.
    nc = self.nc
    drain_inst = nc.sync.drain()
    wait_clock.add_sem_waits(
        drain_inst.ins, ScopedClock({None: tick_clock.global_clock})
    )
    si = drain_inst.ins.sync_info
    waits = list(si.on_wait)
    if len(waits) > 1:
        drain_inst.ins.sync_info = mybir.SyncInfo(
            on_wait=[waits[0]], on_update=list(si.on_update)
        )
        for w in waits[1:]:
            d = nc.sync.drain()
            d.ins.sync_info = mybir.SyncInfo(on_wait=[w], on_update=[])
    nc.all_engine_barrier()
    assert self.sems is not None
    popped = nc._tile_sem_poison_stack.pop()
    assert popped is self._sem_poison
    nc.clear_and_free_semaphores(list(self.sems.allocated().values()))
    nc.all_engine_barrier()


tile.TileContext._drain_and_barrier = _drain_and_barrier_split


def _split_multi_waits(nc, max_waits=1):
    """This container's walrus rejects instructions carrying more than one
    sync wait.  Hoist extra waits onto same-engine NoOps placed just before
    the instruction (waits execute in engine program order, so this is
    semantically identical)."""
    uid = [0]
    for fn in nc.m.functions:
        for bb in fn.blocks:
            insts = bb.instructions
            new = []
            changed = False
            for inst in insts:
                si = getattr(inst, "sync_info", None)
                waits = list(si.on_wait) if si is not None else []
                if len(waits) > max_waits:
                    changed = True
                    n_keep = max_waits
                    for w in waits[:-n_keep]:
                        nop = mybir.InstNoOp(
                            name=f"WSPLIT-{uid[0]}", ins=[], outs=[]
                        )
                        uid[0] += 1
                        nop.engine = inst.engine
                        nop.sync_info = mybir.SyncInfo(
                            on_wait=[w], on_update=[]
                        )
                        new.append(nop)
                    inst.sync_info = mybir.SyncInfo(
                        on_wait=waits[-n_keep:], on_update=list(si.on_update)
                    )
                new.append(inst)
            if changed:
                bb.instructions = new
    return nc


def build_bass():
    nc = bass.Bass(num_devices=N_CORES)
    xw_in = nc.declare_dram_parameter("xw", [XW_ROWS, GD], I8, isOutput=False)
    # full batch-major 7-bit-packed y (+bit-packed f32 row scales),
    # identical on every core via a final AllGather so the host fetches
    # ONE replica
    y = nc.declare_dram_parameter("y", [B * S, YP + 4], I8, isOutput=True)

    KC = D // P               # 16 contraction chunks of 128
    TT = S // 512             # 4 t-tiles of 512
    QI = S // P               # 16 q tiles of 128

    with tile.TileContext(nc) as tc, ExitStack() as ctx:
        # ---- DRAM staging + collectives ----
        dram = ctx.enter_context(tc.tile_pool(name="dram", bufs=1, space="DRAM"))
        xw_loc = dram.tile([XW_ROWS, GD], I8, tag="xw_loc")
        xs_g = dram.tile([4 * BATCH_BLK, GD], I8, tag="xs_g")
        w_g = dram.tile([2 * (XW_ROWS - R_W), GD], I8, tag="w_g", name="w_g")
        y_part = dram.tile([S, D], F32, tag="y_part")
        y_red = dram.tile([GD, D], F32, tag="y_red")

        nc.sync.dma_start(xw_loc[:], xw_in[:])
        nc.gpsimd.collective_compute(
            "AllGather", mybir.AluOpType.bypass,
            replica_groups=GROUPS_BATCH,
            ins=[xw_loc[0:BATCH_BLK, :].opt()], outs=[xs_g.opt()],
        )
        nc.gpsimd.collective_compute(
            "AllGather", mybir.AluOpType.bypass,
            replica_groups=GROUPS_PAIR,
            ins=[xw_loc[R_W:XW_ROWS, :].opt()], outs=[w_g.opt()],
        )
        # weight-half h (0/1) of the pair lives at w_g rows h*4096..
        WH = XW_ROWS - R_W        # 4096 rows per half

        const = ctx.enter_context(tc.tile_pool(name="const", bufs=1))
        maskt = const.tile([P, P], F32)
        make_causal_mask(nc, maskt, mask_val=-1e9)
        ident = const.tile([P, P], BF16)
        make_identity(nc, ident)
        scl_sb = const.tile([P, SCL_C], F32)
        nc.sync.dma_start(
            scl_sb[:], xw_loc[R_ACTS:R_ACTS + P, 0:4 * SCL_C].bitcast(F32)
        )
        sclg_sb = []
        for tt in range(TT):
            t = const.tile([P, SCL_C], F32, tag=f"sclg{tt}")
            r = tt * BATCH_BLK + R_ACTS
            nc.sync.dma_start(t[:], xs_g[r:r + P, 0:4 * SCL_C].bitcast(F32))
            sclg_sb.append(t)
        bq_sb = scl_sb[:, 52:52 + GH]
        bk_sb = scl_sb[:, 56:56 + GH]
        # int8 constants for the 7-bit bit-pack (walrus requires integer-
        # typed operands for bitvec ALU ops, so immediates can't be used)
        c7f = const.tile([P, 1], I8, tag="c7f")
        nc.vector.memset(c7f[:], 0x7F)
        c80 = const.tile([P, 1], I8, tag="c80")
        nc.vector.memset(c80[:], -128)
        csh = []
        for i in range(7):
            t = const.tile([P, 1], I8, tag=f"csh{i}")
            nc.vector.memset(t[:], 7 - i)
            csh.append(t)

        # resident weights: int8 converted to fp16 (q/k) / bf16 (v, wo);
        # integer values are exact in 16-bit floats.
        wpool = ctx.enter_context(tc.tile_pool(name="weights", bufs=1))
        wst = ctx.enter_context(tc.tile_pool(name="wst", bufs=6))
        wq_sb, wk_sb, wv_sb = [], [], []
        for i, (name, lst, dt_) in enumerate((
            ("wq", wq_sb, FP16),
            ("wk", wk_sb, FP16),
            ("wv", wv_sb, BF16),
        )):
            for kc in range(KC):
                st = wst.tile([P, GD], I8, tag="wst")
                r = (kc // 8) * WH + i * (D // 2) + (kc % 8) * P
                nc.sync.dma_start(st[:], w_g[r:r + P, :])
                t = wpool.tile([P, GD], dt_, name=f"{name}{kc}", tag=f"{name}{kc}")
                nc.vector.tensor_copy(t[:], st[:])
                lst.append(t)
        # wo: 16 tiles [128, 512] indexed [hb][oc]; gathered tile (hb, oc)
        # sits at w_g row (hb//2)*WH + 3072 + ((hb%2)*4 + oc)*128
        wo_sb = []
        for hb in range(GH):
            row = []
            for oc in range(TT):
                st = wst.tile([P, GD], I8, tag="wst")
                r = (hb // 2) * WH + 3 * (D // 2) + ((hb % 2) * 4 + oc) * P
                nc.sync.dma_start(st[:], w_g[r:r + P, :])
                t = wpool.tile([P, GD], BF16, name=f"woc{hb}_{oc}",
                               tag=f"wo{hb}_{oc}")
                nc.scalar.activation(
                    t[:], st[:], mybir.ActivationFunctionType.Identity,
                    scale=scl_sb[:, 48 + hb:49 + hb],
                )
                row.append(t)
            wo_sb.append(row)

        # persistent activations
        act = ctx.enter_context(tc.tile_pool(name="acts", bufs=1))
        qT_sb = [act.tile([P, S], FP16, name=f"qT{h}", tag=f"qT{h}") for h in range(GH)]
        kT_sb = [act.tile([P, S], FP16, name=f"kT{h}", tag=f"kT{h}") for h in range(GH)]
        v_sb = [act.tile([P, GD], BF16, name=f"v{i}", tag=f"v{i}") for i in range(QI)]

        ctxA = ExitStack()
        xin = ctxA.enter_context(tc.tile_pool(name="xin", bufs=24))
        xdq = ctxA.enter_context(tc.tile_pool(name="xdq", bufs=24))
        ps512 = ctx.enter_context(
            tc.tile_pool(name="ps512", bufs=4, space="PSUM")
        )

        # xs_g row offset for (column-block tt, tensor j, contraction chunk kc)
        def _xrow(tt, j, kc):
            return tt * BATCH_BLK + j * D + kc * P

        # ---- Q^T / K^T projections: out [dq=512, S] in fp16 ----
        for j, (w_sb, out_tiles, b_tile, scale) in enumerate((
            (wq_sb, qT_sb, bq_sb, SCALE),
            (wk_sb, kT_sb, bk_sb, 1.0),
        )):
            for tt in range(TT):
                xch = []
                for kc in range(KC):
                    ti = xin.tile([P, 512], I8, tag="xin")
                    r = _xrow(tt, j, kc)
                    nc.sync.dma_start(ti[:], xs_g[r:r + P, :])
                    td = xdq.tile([P, 512], FP16, tag="xdq")
                    c = j * KC + kc
                    nc.scalar.activation(
                        td[:], ti[:], mybir.ActivationFunctionType.Identity,
                        scale=sclg_sb[tt][:, c:c + 1],
                    )
                    xch.append(td)
                for dt in range(GH):
                    ps = ps512.tile([P, 512], F32, tag="ps512")
                    for kc in range(KC):
                        nc.tensor.matmul(
                            ps[:],
                            lhsT=w_sb[kc][:, dt * P:(dt + 1) * P],
                            rhs=xch[kc][:],
                            start=(kc == 0),
                            stop=(kc == KC - 1),
                        )
                    # evict: out = (psum + b) * scale, bias pre-scaled on host
                    nc.scalar.activation(
                        out_tiles[dt][:, tt * 512:(tt + 1) * 512],
                        ps[:],
                        mybir.ActivationFunctionType.Identity,
                        bias=b_tile[:, dt:dt + 1],
                        scale=scale,
                    )

        # ---- V projection: out [S, dv=512] in bf16, no bias (folded) ----
        for ttg in range(TT):
            xch = []
            for kc in range(KC):
                ti = xin.tile([P, 512], I8, tag="xin")
                r = _xrow(ttg, 2, kc)
                nc.sync.dma_start(ti[:], xs_g[r:r + P, :])
                td = xdq.tile([P, 512], BF16, tag="xdq")
                c = 2 * KC + kc
                nc.scalar.activation(
                    td[:], ti[:], mybir.ActivationFunctionType.Identity,
                    scale=sclg_sb[ttg][:, c:c + 1],
                )
                xch.append(td)
            for sub in range(4):
                ps = ps512.tile([P, 512], F32, tag="ps512")
                for kc in range(KC):
                    nc.tensor.matmul(
                        ps[:],
                        lhsT=xch[kc][:, sub * P:(sub + 1) * P],
                        rhs=wv_sb[kc][:],
                        start=(kc == 0),
                        stop=(kc == KC - 1),
                    )
                nc.vector.tensor_copy(v_sb[ttg * 4 + sub][:], ps[:])

        ctxA.close()

        # ---- attention + output projection, per q tile ----
        ppool = ctx.enter_context(tc.tile_pool(name="p", bufs=2))
        spool = ctx.enter_context(tc.tile_pool(name="sums", bufs=8))
        ps_t = ctx.enter_context(tc.tile_pool(name="ps_t", bufs=2, space="PSUM"))
        ps_o = ctx.enter_context(tc.tile_pool(name="ps_o", bufs=2, space="PSUM"))
        ptp_pool = ctx.enter_context(tc.tile_pool(name="pt", bufs=3))
        at_pool = ctx.enter_context(tc.tile_pool(name="at", bufs=5))
        attn_pool = ctx.enter_context(tc.tile_pool(name="attn", bufs=2))
        ypool = ctx.enter_context(tc.tile_pool(name="ysb", bufs=3))

        for qi in range(QI):
            kv_len = (qi + 1) * P
            nchunks = (kv_len + 511) // 512
            attn_t = attn_pool.tile([P, GD], BF16, tag="attn")
            for h in range(GH):
                p_t = ppool.tile([P, S], BF16, tag="p")
                sums = spool.tile([P, 4], F32, tag="sums")
                for c in range(nchunks):
                    n = min(512, kv_len - c * 512)
                    ps = ps512.tile([P, 512], F32, tag="ps512")
                    nc.tensor.matmul(
                        ps[:, :n],
                        lhsT=qT_sb[h][:, qi * P:(qi + 1) * P],
                        rhs=kT_sb[h][:, c * 512:c * 512 + n],
                        start=True,
                        stop=True,
                    )
                    if c == nchunks - 1:
                        nc.vector.tensor_add(
                            ps[:, n - P:n], ps[:, n - P:n], maskt[:]
                        )
                    nc.scalar.activation(
                        p_t[:, c * 512:c * 512 + n],
                        ps[:, :n],
                        mybir.ActivationFunctionType.Exp,
                        accum_out=sums[:, c:c + 1],
                    )
                tot = spool.tile([P, 1], F32, tag="tot")
                nc.vector.reduce_sum(
                    tot[:], sums[:, :nchunks], axis=mybir.AxisListType.X
                )
                rec = spool.tile([P, 1], F32, tag="rec")
                nc.vector.reciprocal(rec[:], tot[:])

                po = ps_o.tile([P, P], F32)
                pts = {}

                def _pv_transpose(kb):
                    ptp = ps_t.tile([P, P], BF16, tag="ptp")
                    nc.tensor.transpose(
                        ptp[:], p_t[:, kb * P:(kb + 1) * P], ident[:]
                    )
                    s = ptp_pool.tile([P, P], BF16, tag="pt")
                    nc.vector.tensor_copy(s[:], ptp[:])
                    pts[kb] = s

                # pipeline transposes one block ahead of the PV matmuls so
                # the PE never waits on the DVE copy of the current block
                _pv_transpose(0)
                for kb in range(qi + 1):
                    if kb + 1 <= qi:
                        _pv_transpose(kb + 1)
                    nc.tensor.matmul(
                        po[:],
                        lhsT=pts.pop(kb)[:],
                        rhs=v_sb[kb][:, h * P:(h + 1) * P],
                        start=(kb == 0),
                        stop=(kb == qi),
                    )
                nc.vector.tensor_scalar_mul(
                    attn_t[:, h * P:(h + 1) * P], po[:], rec[:]
                )

            # output projection for this q tile -> partial y in DRAM
            ats = []
            for hb in range(GH):
                atp = ps_t.tile([P, P], BF16, tag="ptp")
                nc.tensor.transpose(
                    atp[:], attn_t[:, hb * P:(hb + 1) * P], ident[:]
                )
                a = at_pool.tile([P, P], BF16, tag="at")
                nc.vector.tensor_copy(a[:], atp[:])
                ats.append(a)
            for oc in range(TT):
                ps = ps512.tile([P, 512], F32, tag="ps512")
                for hb in range(GH):
                    nc.tensor.matmul(
                        ps[:],
                        lhsT=ats[hb][:],
                        rhs=wo_sb[hb][oc][:],
                        start=(hb == 0),
                        stop=(hb == GH - 1),
                    )
                ysb = ypool.tile([P, 512], F32, tag="y")
                nc.scalar.copy(ysb[:], ps[:])
                nc.sync.dma_start(
                    y_part[qi * P:(qi + 1) * P, oc * 512:(oc + 1) * 512],
                    ysb[:],
                )

        # ---- on-device reduction over the batch group ----
        nc.gpsimd.collective_compute(
            "ReduceScatter", mybir.AluOpType.add,
            replica_groups=GROUPS_BATCH,
            ins=[y_part.opt()], outs=[y_red.opt()],
        )
        # 7-bit-quantize rows through SBUF to shrink the device->host
        # bytes: per-row scale s = rowmax(|y|)/62, emit round(y/s) in
        # [-62,62], bit-pack 8 values into 7 bytes (v7's bits ride in the
        # spare top bits of bytes 0..6); f32 scale in the last 4 columns.
        y_q8 = dram.tile([GD, YP + 4], I8, tag="y_q8", name="y_q8")
        ycvt = ctx.enter_context(tc.tile_pool(name="ycvt", bufs=2))
        for r in range(GD // P):
            tf = ycvt.tile([P, D], F32, tag="ycvt_f")
            nc.sync.dma_start(tf[:], y_red[r * P:(r + 1) * P, :])
            mx = ycvt.tile([P, 1], F32, tag="ymx")
            nc.vector.tensor_reduce(
                mx[:], tf[:], axis=mybir.AxisListType.X,
                op=mybir.AluOpType.max, apply_absolute_value=True,
            )
            sc = ycvt.tile([P, 1], F32, tag="ysc")
            nc.vector.tensor_scalar_mul(sc[:], mx[:], 1.0 / 62.0)
            nc.sync.dma_start(
                y_q8[r * P:(r + 1) * P, YP:YP + 4], sc[:].bitcast(I8)
            )
            rcp = ycvt.tile([P, 1], F32, tag="yrcp")
            nc.vector.reciprocal(rcp[:], sc[:])
            tq = ycvt.tile([P, D], F32, tag="ycvt_q")
            nc.vector.tensor_scalar_mul(tq[:], tf[:], rcp[:])
            t8 = ycvt.tile([P, D], I8, tag="ycvt8")
            nc.vector.tensor_copy(t8[:], tq[:])
            tp = ycvt.tile([P, YP], I8, tag="ycvtp")
            tb = ycvt.tile([P, D // 8], I8, tag="ycvtb")
            ta = ycvt.tile([P, D // 8], I8, tag="ycvta")
            for i in range(7):
                # tb = (v7 << (7-i)) & 0x80   (bit i of v7 -> bit 7)
                nc.vector.tensor_scalar(
                    tb[:], t8[:, 7::8], csh[i][:], c80[:],
                    op0=mybir.AluOpType.logical_shift_left,
                    op1=mybir.AluOpType.bitwise_and,
                )
                # packed byte i of each group = (v_i & 0x7F) | tb
                nc.vector.tensor_scalar(
                    ta[:], t8[:, i::8], c7f[:], None,
                    op0=mybir.AluOpType.bitwise_and,
                )
                nc.vector.tensor_tensor(
                    tp[:, i::7], ta[:], tb[:],
                    op=mybir.AluOpType.bitwise_or,
                )
            nc.sync.dma_start(y_q8[r * P:(r + 1) * P, 0:YP], tp[:])
        # gather all 8 slices (batch-major) so every core holds full y
        y_gath = dram.tile([B * S, YP + 4], I8, tag="y_gath", name="y_gath")
        nc.gpsimd.collective_compute(
            "AllGather", mybir.AluOpType.bypass,
            replica_groups=[[0, 1, 2, 3, 4, 5, 6, 7]],
            ins=[y_q8.opt()], outs=[y_gath.opt()],
        )
        nc.sync.dma_start(y[:], y_gath[:])
    _split_multi_waits(nc)
    return nc


# ---------------- host-side runner ----------------

_NC_CACHE = None
_RUNNER = None
_last_in_maps = None


class _Runner:
    """Replicates concourse.bass_utils.run_bass_kernel_spmd's axon/PJRT
    path, but caches the jitted executable across calls (the library
    rebuilds + reloads it every call), skips the donated zero output
    buffers (this kernel writes every output element), and deletes
    stale device buffers to keep the axon tunnel memory-stable.

    Inputs are taken as a dict of already-concatenated global arrays
    (shape [8 * per_core_rows, ...]) keyed by parameter name."""

    def __init__(self, nc, n_cores):
        import jax
        from jax.experimental.shard_map import shard_map
        from jax.sharding import Mesh, PartitionSpec
        from concourse import bass2jax
        from concourse import mybir as _mybir

        bass2jax.install_neuronx_cc_hook()
        self._jax = jax
        self.n_cores = n_cores
        partition_name = (
            nc.partition_id_tensor.name if nc.partition_id_tensor else None
        )
        in_names, out_names, out_avals = [], [], []
        for alloc in nc.m.functions[0].allocations:
            if not isinstance(alloc, _mybir.MemoryLocationSet):
                continue
            name = alloc.memorylocations[0].name
            if alloc.kind == "ExternalInput":
                if name != partition_name:
                    in_names.append(name)
            elif alloc.kind == "ExternalOutput":
                out_names.append(name)
                out_avals.append(
                    jax.core.ShapedArray(
                        tuple(alloc.tensor_shape), _mybir.dt.np(alloc.dtype)
                    )
                )
        self.in_names = in_names
        self.out_names = out_names
        self.out_avals = out_avals
        in_names_all = list(in_names)
        if partition_name is not None:
            in_names_all.append(partition_name)

        def _body(*args):
            operands = list(args)
            if partition_name is not None:
                operands.append(bass2jax.partition_id_tensor())
            outs = bass2jax._bass_exec_p.bind(
                *operands,
                out_avals=tuple(out_avals),
                in_names=tuple(in_names_all),
                out_names=tuple(out_names),
                lowering_input_output_aliases=(),
                sim_require_finite=True,
                sim_require_nnan=True,
                nc=nc,
            )
            return tuple(outs)

        devices = jax.devices()[:n_cores]
        assert len(devices) == n_cores
        mesh = Mesh(np.asarray(devices), ("core",))
        in_specs = (PartitionSpec("core"),) * len(in_names)
        # every core writes the identical full y (final on-device
        # AllGather), so the output is replicated: the host fetches a
        # single replica instead of 8 shards.
        out_specs = (PartitionSpec(),) * len(out_names)
        self._fn = jax.jit(
            shard_map(
                _body, mesh=mesh, in_specs=in_specs, out_specs=out_specs,
                check_rep=False,
            ),
            keep_unused=True,
        )

    def __call__(self, arrs):
        # no explicit .delete(): dropping the jax arrays after the host
        # copy lets PJRT free the buffers asynchronously, which measures
        # ~15-20 ms cheaper per call than a synchronous delete here.
        out_arrs = self._fn(*[arrs[name] for name in self.in_names])
        return {
            name: np.asarray(o)
            for name, o in zip(self.out_names, out_arrs)
        }


def _get_runner():
    global _NC_CACHE, _RUNNER
    if _RUNNER is None:
        _NC_CACHE = build_bass()
        _RUNNER = _Runner(_NC_CACHE, N_CORES)
    return _RUNNER


def _quant_rows(x, levels=127.0):
    """Per-row absmax int8 quantization of a 2-D array. Returns (i8, s)."""
    s = np.abs(x).max(axis=1) / levels
    s[s == 0] = 1.0
    q = np.rint(x / s[:, None]).astype(np.int8)
    return q, s


def _prep_inputs(inputs):
    """Build the globally-concatenated per-parameter arrays directly."""
    query = np.asarray(inputs["query"], np.float32)
    key = np.asarray(inputs["key"], np.float32)
    value = np.asarray(inputs["value"], np.float32)
    Wq = np.asarray(inputs["Wq"], np.float32)
    bq = np.asarray(inputs["bq"], np.float32)
    Wk = np.asarray(inputs["Wk"], np.float32)
    bk = np.asarray(inputs["bk"], np.float32)
    Wv = np.asarray(inputs["Wv"], np.float32)
    Wo = np.asarray(inputs["Wo"], np.float32)

    # weights: int8 with per-contraction-row (= per input-column) scales
    # over the full output dim, so scales are head-group independent.
    w_i8t, w_s = {}, {}
    for nm, W in (("q", Wq), ("k", Wk), ("v", Wv), ("o", Wo)):
        s = np.abs(W).max(axis=0) / 127.0
        s[s == 0] = 1.0
        w_i8t[nm] = np.rint(W.T / s[:, None]).astype(np.int8)  # [d_in, d_out]
        w_s[nm] = s

    xw = np.zeros((N_CORES * XW_ROWS, GD), np.int8)

    for c in range(N_CORES):
        b, g, hb = c // 4, c % 4, c // 4
        gsl = slice(GD * g, GD * (g + 1))
        r0 = c * XW_ROWS
        scl = np.zeros((P, SCL_C), np.float32)
        for j, (x, wn) in enumerate(((query, "q"), (key, "k"), (value, "v"))):
            blk = x[b, gsl, :].T              # [D, 512] feature rows
            qi8, s = _quant_rows(blk)
            xw[r0 + j * D:r0 + (j + 1) * D] = qi8
            # combined scale = act scale * matching weight scale, laid out
            # [128, 16] with partition p <-> d = kc*128+p
            scl[:, j * KC_COLS:(j + 1) * KC_COLS] = (
                (s * w_s[wn]).reshape(D // P, P).T
            )
        # wo scales: full 512 rows of this group as 4 chunks of 128
        scl[:, 48:52] = w_s["o"][gsl].reshape(GH, P).T
        scl[:, 52:56] = (bq[gsl] * SCALE).reshape(GH, P).T
        scl[:, 56:60] = bk[gsl].reshape(GH, P).T
        xw[r0 + R_ACTS:r0 + R_ACTS + P, 0:4 * SCL_C] = scl.view(np.int8)
        wsl = slice((D // 2) * hb, (D // 2) * (hb + 1))
        w0 = r0 + R_W
        xw[w0 + 0 * (D // 2):w0 + 1 * (D // 2)] = w_i8t["q"][wsl, gsl]
        xw[w0 + 1 * (D // 2):w0 + 2 * (D // 2)] = w_i8t["k"][wsl, gsl]
        xw[w0 + 2 * (D // 2):w0 + 3 * (D // 2)] = w_i8t["v"][wsl, gsl]
        # wo half: 8 partition-friendly [128, 512] tiles (lc, oc)
        w1 = w0 + 3 * (D // 2)
        for lc in range(2):
            for oc in range(4):
                a0 = GD * g + (GD // 2) * hb + lc * P
                xw[w1 + (lc * 4 + oc) * P:w1 + (lc * 4 + oc + 1) * P] = (
                    w_i8t["o"][a0:a0 + P, oc * 512:(oc + 1) * 512]
                )

    return {"xw": xw}


KC_COLS = D // P  # 16 scale columns per tensor in scl


def _gather(results, bo_eff):
    yi8 = results["y"]                       # [B*S, YP+4] batch-major
    sc = yi8[:, YP:YP + 4].copy().view(np.float32)
    u = yi8[:, :YP].view(np.uint8)
    v = np.empty((B * S, D), np.float32)
    v7 = np.zeros((B * S, D // 8), np.uint8)
    for i in range(7):
        pi = u[:, i::7]
        v[:, i::8] = (((pi & 0x7F) ^ 0x40).astype(np.int16) - 64).astype(
            np.float32
        )
        v7 |= (pi >> 7).astype(np.uint8) << i
    v[:, 7::8] = ((v7 ^ 0x40).astype(np.int16) - 64).astype(np.float32)
    return (v * sc + bo_eff[None, :]).reshape(B, S, D)


def kernel(**inputs):
    global _last_in_maps
    bo = np.asarray(inputs["bo"], np.float32)
    bv = np.asarray(inputs["bv"], np.float32)
    Wo = np.asarray(inputs["Wo"], np.float32)
    bo_eff = bo + bv @ Wo.T   # V bias folds through softmax-normalized P
    arrs = _prep_inputs(inputs)
    _last_in_maps = arrs
    runner = _get_runner()
    results = runner(arrs)
    return _gather(results, bo_eff)


# revision 40
# speedup vs baseline: 1.1291x; 1.0616x over previous
"""Trainium2 Bass kernel for KV-cached (causal) multi-head attention.

Full module: y = softmax(mask(QK^T/sqrt(hd))) V  -> out_proj, with
Q/K/V = linear projections of query/key/value inputs.

Shapes (hardcoded): B=2, S=2048, D=2048, H=16 heads, hd=128.

Sharding (8 NeuronCores): core c handles batch b=c//4 and head group
g=c%4 (4 heads = 512 dims).  The host<->device tunnel is the wall-time
bottleneck (~50-80 MB/s shared pipe), so all bulk traffic is int8:

  - activations: core c receives the c%4-th 512-row slice of its
    batch's query/key/value, transposed to [D, 512] and int8-quantized
    per feature row (scale = absmax/127 over the 512 seq positions).
    An AllGather over the batch group {4b..4b+3} rebuilds the full
    [D, S] transposed activations as 4 column blocks.
  - weights: int8 with per-contraction-row scales taken over the FULL
    output row (so the scale is head-group independent); cores c and
    c+4 need identical TP weight slices, so each receives half and an
    AllGather over pairs {c, c+4} rebuilds them.  Because both the
    activation scale and the weight scale are per-contraction-dim, the
    host pre-multiplies them into a single combined scale applied to
    the activation tiles on device; the int8 weights are then used
    EXACTLY (integers <=127 are exact in fp16/bf16).
  - the V bias is folded into the output bias on the host
    (softmax rows sum to 1, so P(V + bv) = PV + bv after
    normalization => bo' = bo + bv @ Wo.T), removing it from the
    device entirely.
  - output: partial out-projections are summed on device with a
    ReduceScatter over the batch group; each core returns only its
    512-row slice of y, int8-quantized with per-row f32 scales that
    are bit-packed into 4 extra int8 columns (single output tensor).

On-device layout (fp16 matmuls on the Q/K path for extra mantissa,
bf16 elsewhere, fp32 PSUM accumulation):
  - Q^T, K^T computed as [dq, S] (head dim on partitions) so that
    scores = Q^T.T @ K^T needs no on-device transposes
  - V computed as [S, dv]
  - softmax per q-row (partition) along free kv axis; exp on ScalarE
    with fused per-chunk row-sums (accum_out); causal handled by
    skipping kv blocks beyond the diagonal + one additive mask tile
    on the diagonal 128x128 block
  - P^T for the PV matmul via PE-mode transposes of 128x128 blocks
  - attention output [q, hd] re-transposed per 128-block to feed the
    output projection as lhsT
"""

import sys

for _p in ("/opt/trn_rl_repo",):
    if _p not in sys.path:
        sys.path.insert(0, _p)

from contextlib import ExitStack

import numpy as np

import concourse.bass as bass
import concourse.mybir as mybir
import concourse.tile as tile
from concourse.vector_clock import ScopedClock
from concourse.masks import make_causal_mask, make_identity

BF16 = mybir.dt.bfloat16
FP16 = mybir.dt.float16
F32 = mybir.dt.float32
I8 = mybir.dt.int8

B, S, D = 2, 2048, 2048
NH, HD = 16, 128          # total heads, head dim
GH = 4                    # heads per core
GD = GH * HD              # 512 dims per core
P = 128
SCALE = 1.0 / np.sqrt(HD)
N_CORES = 8

GROUPS_BATCH = [[0, 1, 2, 3], [4, 5, 6, 7]]   # share one batch's acts
GROUPS_PAIR = [[0, 4], [1, 5], [2, 6], [3, 7]]  # share TP weight slices

# Two packed int8 inputs per core.  xw [XW_ROWS, 512]: vT acts (2048) |
# f32 scale/bias block bit-packed into int8 (128 rows, AllGathered with
# v) | wv half (1024) | wo half as 8 partition-friendly tiles (1024).
# xqk [QK_ROWS, 448]: 7-bit values bit-packed 8-into-7 bytes per row
# (same sign-bit-stuffing as the y output): qT (2048) | kT (2048) |
# wq half (1024) | wk half (1024).
# scl columns: 0..47 combined act*weight scales (j*16+kc), 48..51 wo row
# scales (full 512 of this group as 4 chunks), 52..55 bq*SCALE, 56..59 bk
XW_V = D                         # 2048 v rows
BATCH_BLK = XW_V + P             # 2176 rows gathered over the batch group
R_W = BATCH_BLK                  # weight half rows start here
XW_ROWS = BATCH_BLK + (D // 2) + (D // 2)       # 4224
QK_BBLK = 2 * D                  # 4096 q+k rows gathered over the group
QK_ROWS = QK_BBLK + 2 * (D // 2)                # 6144
PW = (512 // 8) * 7              # 448 packed bytes per 512 values
SCL_C = 60
# y rows travel as 7-bit values bit-packed 8-into-7 bytes: byte i of each
# group carries v_i's low 7 bits, and bit 7 of bytes 0..6 carries bit i
# of v_7.  2048 cols -> 1792 packed bytes + 4 bytes of f32 row scale.
YP = (D // 8) * 7                # 1792


def _drain_and_barrier_split(self, tick_clock, wait_clock):
    # The walrus build in this container rejects a Drain carrying more
    # than one sync wait ("Too many sync wait commands").  Semantically
    # equivalent: chain one drain per wait on the sync engine.
    nc = self.nc
    drain_inst = nc.sync.drain()
    wait_clock.add_sem_waits(
        drain_inst.ins, ScopedClock({None: tick_clock.global_clock})
    )
    si = drain_inst.ins.sync_info
    waits = list(si.on_wait)
    if len(waits) > 1:
        drain_inst.ins.sync_info = mybir.SyncInfo(
            on_wait=[waits[0]], on_update=list(si.on_update)
        )
        for w in waits[1:]:
            d = nc.sync.drain()
            d.ins.sync_info = mybir.SyncInfo(on_wait=[w], on_update=[])
    nc.all_engine_barrier()
    assert self.sems is not None
    popped = nc._tile_sem_poison_stack.pop()
    assert popped is self._sem_poison
    nc.clear_and_free_semaphores(list(self.sems.allocated().values()))
    nc.all_engine_barrier()


tile.TileContext._drain_and_barrier = _drain_and_barrier_split


def _split_multi_waits(nc, max_waits=1):
    """This container's walrus rejects instructions carrying more than one
    sync wait.  Hoist extra waits onto same-engine NoOps placed just before
    the instruction (waits execute in engine program order, so this is
    semantically identical)."""
    uid = [0]
    for fn in nc.m.functions:
        for bb in fn.blocks:
            insts = bb.instructions
            new = []
            changed = False
            for inst in insts:
                si = getattr(inst, "sync_info", None)
                waits = list(si.on_wait) if si is not None else []
                if len(waits) > max_waits:
                    changed = True
                    n_keep = max_waits
                    for w in waits[:-n_keep]:
                        nop = mybir.InstNoOp(
                            name=f"WSPLIT-{uid[0]}", ins=[], outs=[]
                        )
                        uid[0] += 1
                        nop.engine = inst.engine
                        nop.sync_info = mybir.SyncInfo(
                            on_wait=[w], on_update=[]
                        )
                        new.append(nop)
                    inst.sync_info = mybir.SyncInfo(
                        on_wait=waits[-n_keep:], on_update=list(si.on_update)
                    )
                new.append(inst)
            if changed:
                bb.instructions = new
    return nc


def build_bass():
    nc = bass.Bass(num_devices=N_CORES)
    xw_in = nc.declare_dram_parameter("xw", [XW_ROWS, GD], I8, isOutput=False)
    xqk_in = nc.declare_dram_parameter("xqk", [QK_ROWS, PW], I8, isOutput=False)
    # full batch-major 7-bit-packed y (+bit-packed f32 row scales),
    # identical on every core via a final AllGather so the host fetches
    # ONE replica
    y = nc.declare_dram_parameter("y", [B * S, YP + 4], I8, isOutput=True)

    KC = D // P               # 16 contraction chunks of 128
    TT = S // 512             # 4 t-tiles of 512
    QI = S // P               # 16 q tiles of 128

    with tile.TileContext(nc) as tc, ExitStack() as ctx:
        # ---- DRAM staging + collectives ----
        dram = ctx.enter_context(tc.tile_pool(name="dram", bufs=1, space="DRAM"))
        xw_loc = dram.tile([XW_ROWS, GD], I8, tag="xw_loc")
        xqk_loc = dram.tile([QK_ROWS, PW], I8, tag="xqk_loc")
        xs_g = dram.tile([4 * BATCH_BLK, GD], I8, tag="xs_g")
        w_g = dram.tile([2 * (XW_ROWS - R_W), GD], I8, tag="w_g", name="w_g")
        xsqk_g = dram.tile([4 * QK_BBLK, PW], I8, tag="xsqk_g", name="xsqk_g")
        wqk_g = dram.tile([2 * D, PW], I8, tag="wqk_g", name="wqk_g")
        y_part = dram.tile([S, D], F32, tag="y_part")
        y_red = dram.tile([GD, D], F32, tag="y_red")

        nc.sync.dma_start(xw_loc[:], xw_in[:])
        nc.sync.dma_start(xqk_loc[:], xqk_in[:])
        nc.gpsimd.collective_compute(
            "AllGather", mybir.AluOpType.bypass,
            replica_groups=GROUPS_BATCH,
            ins=[xw_loc[0:BATCH_BLK, :].opt()], outs=[xs_g.opt()],
        )
        nc.gpsimd.collective_compute(
            "AllGather", mybir.AluOpType.bypass,
            replica_groups=GROUPS_PAIR,
            ins=[xw_loc[R_W:XW_ROWS, :].opt()], outs=[w_g.opt()],
        )
        nc.gpsimd.collective_compute(
            "AllGather", mybir.AluOpType.bypass,
            replica_groups=GROUPS_BATCH,
            ins=[xqk_loc[0:QK_BBLK, :].opt()], outs=[xsqk_g.opt()],
        )
        nc.gpsimd.collective_compute(
            "AllGather", mybir.AluOpType.bypass,
            replica_groups=GROUPS_PAIR,
            ins=[xqk_loc[QK_BBLK:QK_ROWS, :].opt()], outs=[wqk_g.opt()],
        )
        # weight-half h (0/1) of the pair lives at w_g rows h*WH..
        WH = XW_ROWS - R_W        # 2048 rows per half (wv + wo)

        const = ctx.enter_context(tc.tile_pool(name="const", bufs=1))
        maskt = const.tile([P, P], F32)
        make_causal_mask(nc, maskt, mask_val=-1e9)
        ident = const.tile([P, P], BF16)
        make_identity(nc, ident)
        scl_sb = const.tile([P, SCL_C], F32)
        nc.sync.dma_start(
            scl_sb[:], xw_loc[XW_V:XW_V + P, 0:4 * SCL_C].bitcast(F32)
        )
        sclg_sb = []
        for tt in range(TT):
            t = const.tile([P, SCL_C], F32, tag=f"sclg{tt}")
            r = tt * BATCH_BLK + XW_V
            nc.sync.dma_start(t[:], xs_g[r:r + P, 0:4 * SCL_C].bitcast(F32))
            sclg_sb.append(t)
        bq_sb = scl_sb[:, 52:52 + GH]
        bk_sb = scl_sb[:, 56:56 + GH]
        c40 = const.tile([P, 1], I8, tag="c40")
        nc.vector.memset(c40[:], 0x40)
        c40f = const.tile([P, 1], F32, tag="c40f")
        nc.vector.memset(c40f[:], 64.0)
        # f32 multipliers that turn (b & 0x80) = 0/-128 into 0/2^i
        cmul = []
        for i in range(7):
            t = const.tile([P, 1], F32, tag=f"cmul{i}")
            nc.vector.memset(t[:], -(2.0 ** i) / 128.0)
            cmul.append(t)
        # int8 constants for the 7-bit bit-pack (walrus requires integer-
        # typed operands for bitvec ALU ops, so immediates can't be used)
        c7f = const.tile([P, 1], I8, tag="c7f")
        nc.vector.memset(c7f[:], 0x7F)
        c80 = const.tile([P, 1], I8, tag="c80")
        nc.vector.memset(c80[:], -128)
        csh = []
        for i in range(7):
            t = const.tile([P, 1], I8, tag=f"csh{i}")
            nc.vector.memset(t[:], 7 - i)
            csh.append(t)

        # 7-bit unpack: src [P, 448] packed -> dst [P, 512] int8.  NOTE
        # logical_shift_right on int8 lanes is ARITHMETIC on this DVE
        # (sign-fills), so the top bit is extracted via & 0x80 then an
        # f32 multiply by -(2^i)/128 instead of any right shift.
        tb_pool = ctx.enter_context(tc.tile_pool(name="tb7", bufs=4))

        def _sext7(dst, src):
            # sign-extend a 7-bit value: ((x & 0x7F) ^ 0x40) - 0x40
            nc.vector.tensor_scalar(
                dst, src, c7f[:], c40[:],
                op0=mybir.AluOpType.bitwise_and,
                op1=mybir.AluOpType.bitwise_xor,
            )
            nc.vector.tensor_scalar(
                dst, dst, c40f[:], None, op0=mybir.AluOpType.subtract,
            )

        def _unpack7(dst, src):
            for i in range(7):
                _sext7(dst[:, i::8], src[:, i::7])
            # v_7: bit i rides bit 7 of byte i of each group
            nc.vector.tensor_scalar(
                dst[:, 7::8], src[:, 0::7], c80[:], None,
                op0=mybir.AluOpType.bitwise_and,
            )
            nc.vector.tensor_scalar(
                dst[:, 7::8], dst[:, 7::8], cmul[0][:], None,
                op0=mybir.AluOpType.mult,
            )
            for i in range(1, 7):
                tb2 = tb_pool.tile([P, 64], I8, tag="tb2")
                nc.vector.tensor_scalar(
                    tb2[:], src[:, i::7], c80[:], None,
                    op0=mybir.AluOpType.bitwise_and,
                )
                nc.vector.tensor_scalar(
                    tb2[:], tb2[:], cmul[i][:], None,
                    op0=mybir.AluOpType.mult,
                )
                nc.vector.tensor_tensor(
                    dst[:, 7::8], dst[:, 7::8], tb2[:],
                    op=mybir.AluOpType.bitwise_or,
                )
            _sext7(dst[:, 7::8], dst[:, 7::8])

        # resident weights: int8 converted to fp16 (q/k, 7-bit-packed on
        # the wire) / bf16 (v, wo); integer values are exact in 16-bit.
        wpool = ctx.enter_context(tc.tile_pool(name="weights", bufs=1))
        wst = ctx.enter_context(tc.tile_pool(name="wst", bufs=6))
        wq_sb, wk_sb, wv_sb = [], [], []
        for i, (name, lst) in enumerate((("wq", wq_sb), ("wk", wk_sb))):
            for kc in range(KC):
                st = wst.tile([P, PW], I8, tag="wstp")
                r = (kc // 8) * D + i * (D // 2) + (kc % 8) * P
                nc.sync.dma_start(st[:], wqk_g[r:r + P, :])
                su = wst.tile([P, GD], I8, tag="wstu")
                _unpack7(su[:], st[:])
                t = wpool.tile([P, GD], FP16, name=f"{name}{kc}", tag=f"{name}{kc}")
                nc.vector.tensor_copy(t[:], su[:])
                lst.append(t)
        for kc in range(KC):
            st = wst.tile([P, GD], I8, tag="wst")
            r = (kc // 8) * WH + (kc % 8) * P
            nc.sync.dma_start(st[:], w_g[r:r + P, :])
            t = wpool.tile([P, GD], BF16, name=f"wv{kc}", tag=f"wv{kc}")
            nc.vector.tensor_copy(t[:], st[:])
            wv_sb.append(t)
        # wo: 16 tiles [128, 512] indexed [hb][oc]; gathered tile (hb, oc)
        # sits at w_g row (hb//2)*WH + 1024 + ((hb%2)*4 + oc)*128
        wo_sb = []
        for hb in range(GH):
            row = []
            for oc in range(TT):
                st = wst.tile([P, GD], I8, tag="wst")
                r = (hb // 2) * WH + (D // 2) + ((hb % 2) * 4 + oc) * P
                nc.sync.dma_start(st[:], w_g[r:r + P, :])
                t = wpool.tile([P, GD], BF16, name=f"woc{hb}_{oc}",
                               tag=f"wo{hb}_{oc}")
                nc.scalar.activation(
                    t[:], st[:], mybir.ActivationFunctionType.Identity,
                    scale=scl_sb[:, 48 + hb:49 + hb],
                )
                row.append(t)
            wo_sb.append(row)

        # persistent activations
        act = ctx.enter_context(tc.tile_pool(name="acts", bufs=1))
        qT_sb = [act.tile([P, S], FP16, name=f"qT{h}", tag=f"qT{h}") for h in range(GH)]
        kT_sb = [act.tile([P, S], FP16, name=f"kT{h}", tag=f"kT{h}") for h in range(GH)]
        v_sb = [act.tile([P, GD], BF16, name=f"v{i}", tag=f"v{i}") for i in range(QI)]

        ctxA = ExitStack()
        xin = ctxA.enter_context(tc.tile_pool(name="xin", bufs=24))
        xu8 = ctxA.enter_context(tc.tile_pool(name="xu8", bufs=8))
        xdq = ctxA.enter_context(tc.tile_pool(name="xdq", bufs=24))
        ps512 = ctx.enter_context(
            tc.tile_pool(name="ps512", bufs=4, space="PSUM")
        )

        # ---- Q^T / K^T projections: out [dq=512, S] in fp16 ----
        for j, (w_sb, out_tiles, b_tile, scale) in enumerate((
            (wq_sb, qT_sb, bq_sb, SCALE),
            (wk_sb, kT_sb, bk_sb, 1.0),
        )):
            for tt in range(TT):
                xch = []
                for kc in range(KC):
                    ti = xin.tile([P, PW], I8, tag="xinp")
                    r = tt * QK_BBLK + j * D + kc * P
                    nc.sync.dma_start(ti[:], xsqk_g[r:r + P, :])
                    tu = xu8.tile([P, 512], I8, tag="xu8")
                    _unpack7(tu[:], ti[:])
                    td = xdq.tile([P, 512], FP16, tag="xdq")
                    c = j * KC + kc
                    nc.scalar.activation(
                        td[:], tu[:], mybir.ActivationFunctionType.Identity,
                        scale=sclg_sb[tt][:, c:c + 1],
                    )
                    xch.append(td)
                for dt in range(GH):
                    ps = ps512.tile([P, 512], F32, tag="ps512")
                    for kc in range(KC):
                        nc.tensor.matmul(
                            ps[:],
                            lhsT=w_sb[kc][:, dt * P:(dt + 1) * P],
                            rhs=xch[kc][:],
                            start=(kc == 0),
                            stop=(kc == KC - 1),
                        )
                    # evict: out = (psum + b) * scale, bias pre-scaled on host
                    nc.scalar.activation(
                        out_tiles[dt][:, tt * 512:(tt + 1) * 512],
                        ps[:],
                        mybir.ActivationFunctionType.Identity,
                        bias=b_tile[:, dt:dt + 1],
                        scale=scale,
                    )

        # ---- V projection: out [S, dv=512] in bf16, no bias (folded) ----
        for ttg in range(TT):
            xch = []
            for kc in range(KC):
                ti = xin.tile([P, 512], I8, tag="xin")
                r = ttg * BATCH_BLK + kc * P
                nc.sync.dma_start(ti[:], xs_g[r:r + P, :])
                td = xdq.tile([P, 512], BF16, tag="xdq")
                c = 2 * KC + kc
                nc.scalar.activation(
                    td[:], ti[:], mybir.ActivationFunctionType.Identity,
                    scale=sclg_sb[ttg][:, c:c + 1],
                )
                xch.append(td)
            for sub in range(4):
                ps = ps512.tile([P, 512], F32, tag="ps512")
                for kc in range(KC):
                    nc.tensor.matmul(
                        ps[:],
                        lhsT=xch[kc][:, sub * P:(sub + 1) * P],
                        rhs=wv_sb[kc][:],
                        start=(kc == 0),
                        stop=(kc == KC - 1),
                    )
                nc.vector.tensor_copy(v_sb[ttg * 4 + sub][:], ps[:])

        ctxA.close()

        # ---- attention + output projection, per q tile ----
        ppool = ctx.enter_context(tc.tile_pool(name="p", bufs=2))
        spool = ctx.enter_context(tc.tile_pool(name="sums", bufs=8))
        ps_t = ctx.enter_context(tc.tile_pool(name="ps_t", bufs=2, space="PSUM"))
        ps_o = ctx.enter_context(tc.tile_pool(name="ps_o", bufs=2, space="PSUM"))
        ptp_pool = ctx.enter_context(tc.tile_pool(name="pt", bufs=3))
        at_pool = ctx.enter_context(tc.tile_pool(name="at", bufs=5))
        attn_pool = ctx.enter_context(tc.tile_pool(name="attn", bufs=2))
        ypool = ctx.enter_context(tc.tile_pool(name="ysb", bufs=3))

        for qi in range(QI):
            kv_len = (qi + 1) * P
            nchunks = (kv_len + 511) // 512
            attn_t = attn_pool.tile([P, GD], BF16, tag="attn")
            for h in range(GH):
                p_t = ppool.tile([P, S], BF16, tag="p")
                sums = spool.tile([P, 4], F32, tag="sums")
                for c in range(nchunks):
                    n = min(512, kv_len - c * 512)
                    ps = ps512.tile([P, 512], F32, tag="ps512")
                    nc.tensor.matmul(
                        ps[:, :n],
                        lhsT=qT_sb[h][:, qi * P:(qi + 1) * P],
                        rhs=kT_sb[h][:, c * 512:c * 512 + n],
                        start=True,
                        stop=True,
                    )
                    if c == nchunks - 1:
                        nc.vector.tensor_add(
                            ps[:, n - P:n], ps[:, n - P:n], maskt[:]
                        )
                    nc.scalar.activation(
                        p_t[:, c * 512:c * 512 + n],
                        ps[:, :n],
                        mybir.ActivationFunctionType.Exp,
                        accum_out=sums[:, c:c + 1],
                    )
                tot = spool.tile([P, 1], F32, tag="tot")
                nc.vector.reduce_sum(
                    tot[:], sums[:, :nchunks], axis=mybir.AxisListType.X
                )
                rec = spool.tile([P, 1], F32, tag="rec")
                nc.vector.reciprocal(rec[:], tot[:])

                po = ps_o.tile([P, P], F32)
                pts = {}

                def _pv_transpose(kb):
                    ptp = ps_t.tile([P, P], BF16, tag="ptp")
                    nc.tensor.transpose(
                        ptp[:], p_t[:, kb * P:(kb + 1) * P], ident[:]
                    )
                    s = ptp_pool.tile([P, P], BF16, tag="pt")
                    nc.vector.tensor_copy(s[:], ptp[:])
                    pts[kb] = s

                # pipeline transposes one block ahead of the PV matmuls so
                # the PE never waits on the DVE copy of the current block
                _pv_transpose(0)
                for kb in range(qi + 1):
                    if kb + 1 <= qi:
                        _pv_transpose(kb + 1)
                    nc.tensor.matmul(
                        po[:],
                        lhsT=pts.pop(kb)[:],
                        rhs=v_sb[kb][:, h * P:(h + 1) * P],
                        start=(kb == 0),
                        stop=(kb == qi),
                    )
                nc.vector.tensor_scalar_mul(
                    attn_t[:, h * P:(h + 1) * P], po[:], rec[:]
                )

            # output projection for this q tile -> partial y in DRAM
            ats = []
            for hb in range(GH):
                atp = ps_t.tile([P, P], BF16, tag="ptp")
                nc.tensor.transpose(
                    atp[:], attn_t[:, hb * P:(hb + 1) * P], ident[:]
                )
                a = at_pool.tile([P, P], BF16, tag="at")
                nc.vector.tensor_copy(a[:], atp[:])
                ats.append(a)
            for oc in range(TT):
                ps = ps512.tile([P, 512], F32, tag="ps512")
                for hb in range(GH):
                    nc.tensor.matmul(
                        ps[:],
                        lhsT=ats[hb][:],
                        rhs=wo_sb[hb][oc][:],
                        start=(hb == 0),
                        stop=(hb == GH - 1),
                    )
                ysb = ypool.tile([P, 512], F32, tag="y")
                nc.scalar.copy(ysb[:], ps[:])
                nc.sync.dma_start(
                    y_part[qi * P:(qi + 1) * P, oc * 512:(oc + 1) * 512],
                    ysb[:],
                )

        # ---- on-device reduction over the batch group ----
        nc.gpsimd.collective_compute(
            "ReduceScatter", mybir.AluOpType.add,
            replica_groups=GROUPS_BATCH,
            ins=[y_part.opt()], outs=[y_red.opt()],
        )
        # 7-bit-quantize rows through SBUF to shrink the device->host
        # bytes: per-row scale s = rowmax(|y|)/62, emit round(y/s) in
        # [-62,62], bit-pack 8 values into 7 bytes (v7's bits ride in the
        # spare top bits of bytes 0..6); f32 scale in the last 4 columns.
        y_q8 = dram.tile([GD, YP + 4], I8, tag="y_q8", name="y_q8")
        ycvt = ctx.enter_context(tc.tile_pool(name="ycvt", bufs=2))
        for r in range(GD // P):
            tf = ycvt.tile([P, D], F32, tag="ycvt_f")
            nc.sync.dma_start(tf[:], y_red[r * P:(r + 1) * P, :])
            mx = ycvt.tile([P, 1], F32, tag="ymx")
            nc.vector.tensor_reduce(
                mx[:], tf[:], axis=mybir.AxisListType.X,
                op=mybir.AluOpType.max, apply_absolute_value=True,
            )
            sc = ycvt.tile([P, 1], F32, tag="ysc")
            nc.vector.tensor_scalar_mul(sc[:], mx[:], 1.0 / 62.0)
            nc.sync.dma_start(
                y_q8[r * P:(r + 1) * P, YP:YP + 4], sc[:].bitcast(I8)
            )
            rcp = ycvt.tile([P, 1], F32, tag="yrcp")
            nc.vector.reciprocal(rcp[:], sc[:])
            tq = ycvt.tile([P, D], F32, tag="ycvt_q")
            nc.vector.tensor_scalar_mul(tq[:], tf[:], rcp[:])
            t8 = ycvt.tile([P, D], I8, tag="ycvt8")
            nc.vector.tensor_copy(t8[:], tq[:])
            tp = ycvt.tile([P, YP], I8, tag="ycvtp")
            tb = ycvt.tile([P, D // 8], I8, tag="ycvtb")
            ta = ycvt.tile([P, D // 8], I8, tag="ycvta")
            for i in range(7):
                # tb = (v7 << (7-i)) & 0x80   (bit i of v7 -> bit 7)
                nc.vector.tensor_scalar(
                    tb[:], t8[:, 7::8], csh[i][:], c80[:],
                    op0=mybir.AluOpType.logical_shift_left,
                    op1=mybir.AluOpType.bitwise_and,
                )
                # packed byte i of each group = (v_i & 0x7F) | tb
                nc.vector.tensor_scalar(
                    ta[:], t8[:, i::8], c7f[:], None,
                    op0=mybir.AluOpType.bitwise_and,
                )
                nc.vector.tensor_tensor(
                    tp[:, i::7], ta[:], tb[:],
                    op=mybir.AluOpType.bitwise_or,
                )
            nc.sync.dma_start(y_q8[r * P:(r + 1) * P, 0:YP], tp[:])
        # gather all 8 slices (batch-major) so every core holds full y
        y_gath = dram.tile([B * S, YP + 4], I8, tag="y_gath", name="y_gath")
        nc.gpsimd.collective_compute(
            "AllGather", mybir.AluOpType.bypass,
            replica_groups=[[0, 1, 2, 3, 4, 5, 6, 7]],
            ins=[y_q8.opt()], outs=[y_gath.opt()],
        )
        nc.sync.dma_start(y[:], y_gath[:])
    _split_multi_waits(nc)
    return nc


# ---------------- host-side runner ----------------

_NC_CACHE = None
_RUNNER = None
_last_in_maps = None


class _Runner:
    """Replicates concourse.bass_utils.run_bass_kernel_spmd's axon/PJRT
    path, but caches the jitted executable across calls (the library
    rebuilds + reloads it every call), skips the donated zero output
    buffers (this kernel writes every output element), and deletes
    stale device buffers to keep the axon tunnel memory-stable.

    Inputs are taken as a dict of already-concatenated global arrays
    (shape [8 * per_core_rows, ...]) keyed by parameter name."""

    def __init__(self, nc, n_cores):
        import jax
        from jax.experimental.shard_map import shard_map
        from jax.sharding import Mesh, PartitionSpec
        from concourse import bass2jax
        from concourse import mybir as _mybir

        bass2jax.install_neuronx_cc_hook()
        self._jax = jax
        self.n_cores = n_cores
        partition_name = (
            nc.partition_id_tensor.name if nc.partition_id_tensor else None
        )
        in_names, out_names, out_avals = [], [], []
        for alloc in nc.m.functions[0].allocations:
            if not isinstance(alloc, _mybir.MemoryLocationSet):
                continue
            name = alloc.memorylocations[0].name
            if alloc.kind == "ExternalInput":
                if name != partition_name:
                    in_names.append(name)
            elif alloc.kind == "ExternalOutput":
                out_names.append(name)
                out_avals.append(
                    jax.core.ShapedArray(
                        tuple(alloc.tensor_shape), _mybir.dt.np(alloc.dtype)
                    )
                )
        self.in_names = in_names
        self.out_names = out_names
        self.out_avals = out_avals
        in_names_all = list(in_names)
        if partition_name is not None:
            in_names_all.append(partition_name)

        def _body(*args):
            operands = list(args)
            if partition_name is not None:
                operands.append(bass2jax.partition_id_tensor())
            outs = bass2jax._bass_exec_p.bind(
                *operands,
                out_avals=tuple(out_avals),
                in_names=tuple(in_names_all),
                out_names=tuple(out_names),
                lowering_input_output_aliases=(),
                sim_require_finite=True,
                sim_require_nnan=True,
                nc=nc,
            )
            return tuple(outs)

        devices = jax.devices()[:n_cores]
        assert len(devices) == n_cores
        mesh = Mesh(np.asarray(devices), ("core",))
        in_specs = (PartitionSpec("core"),) * len(in_names)
        # every core writes the identical full y (final on-device
        # AllGather), so the output is replicated: the host fetches a
        # single replica instead of 8 shards.
        out_specs = (PartitionSpec(),) * len(out_names)
        self._fn = jax.jit(
            shard_map(
                _body, mesh=mesh, in_specs=in_specs, out_specs=out_specs,
                check_rep=False,
            ),
            keep_unused=True,
        )

    def __call__(self, arrs):
        # no explicit .delete(): dropping the jax arrays after the host
        # copy lets PJRT free the buffers asynchronously, which measures
        # ~15-20 ms cheaper per call than a synchronous delete here.
        out_arrs = self._fn(*[arrs[name] for name in self.in_names])
        return {
            name: np.asarray(o)
            for name, o in zip(self.out_names, out_arrs)
        }


def _get_runner():
    global _NC_CACHE, _RUNNER
    if _RUNNER is None:
        _NC_CACHE = build_bass()
        _RUNNER = _Runner(_NC_CACHE, N_CORES)
    return _RUNNER


def _quant_rows(x, levels=127.0):
    """Per-row absmax int8 quantization of a 2-D array. Returns (i8, s)."""
    s = np.abs(x).max(axis=1) / levels
    s[s == 0] = 1.0
    q = np.rint(x / s[:, None]).astype(np.int8)
    return q, s


def _pack7(a):
    """Bit-pack 7-bit int8 values [r, 512] -> [r, 448] bytes: byte i of
    each 8-group keeps v_i's low 7 bits, bit 7 of bytes 0..6 carries bit
    i of v_7 (same format the kernel packs y with on the way out)."""
    u = a.view(np.uint8) & 0x7F
    g = u.reshape(a.shape[0], 64, 8)
    bits = ((g[:, :, 7:] >> np.arange(7, dtype=np.uint8)) & 1) << np.uint8(7)
    return (g[:, :, :7] | bits).reshape(a.shape[0], PW).view(np.int8)


def _prep_inputs(inputs):
    """Build the globally-concatenated per-parameter arrays directly."""
    query = np.asarray(inputs["query"], np.float32)
    key = np.asarray(inputs["key"], np.float32)
    value = np.asarray(inputs["value"], np.float32)
    Wq = np.asarray(inputs["Wq"], np.float32)
    bq = np.asarray(inputs["bq"], np.float32)
    Wk = np.asarray(inputs["Wk"], np.float32)
    bk = np.asarray(inputs["bk"], np.float32)
    Wv = np.asarray(inputs["Wv"], np.float32)
    Wo = np.asarray(inputs["Wo"], np.float32)

    # weights: per-contraction-row (= per input-column) scales over the
    # full output dim, so scales are head-group independent.  Wq/Wk are
    # 7-bit (they ride the bit-packed xqk param), Wv/Wo int8.
    w_i8t, w_s = {}, {}
    for nm, W, lv in (("q", Wq, 63.0), ("k", Wk, 63.0),
                      ("v", Wv, 127.0), ("o", Wo, 127.0)):
        s = np.abs(W).max(axis=0) / lv
        s[s == 0] = 1.0
        w_i8t[nm] = np.rint(W.T / s[:, None]).astype(np.int8)  # [d_in, d_out]
        w_s[nm] = s

    xw = np.zeros((N_CORES * XW_ROWS, GD), np.int8)
    xqk = np.empty((N_CORES * QK_ROWS, PW), np.int8)

    for c in range(N_CORES):
        b, g, hb = c // 4, c % 4, c // 4
        gsl = slice(GD * g, GD * (g + 1))
        r0 = c * XW_ROWS
        q0 = c * QK_ROWS
        scl = np.zeros((P, SCL_C), np.float32)
        for j, (x, wn, lv) in enumerate(
            ((query, "q", 63.0), (key, "k", 63.0), (value, "v", 127.0))
        ):
            blk = x[b, gsl, :].T              # [D, 512] feature rows
            qi8, s = _quant_rows(blk, lv)
            if j < 2:
                xqk[q0 + j * D:q0 + (j + 1) * D] = _pack7(qi8)
            else:
                xw[r0:r0 + D] = qi8
            # combined scale = act scale * matching weight scale, laid out
            # [128, 16] with partition p <-> d = kc*128+p
            scl[:, j * KC_COLS:(j + 1) * KC_COLS] = (
                (s * w_s[wn]).reshape(D // P, P).T
            )
        # wo scales: full 512 rows of this group as 4 chunks of 128
        scl[:, 48:52] = w_s["o"][gsl].reshape(GH, P).T
        scl[:, 52:56] = (bq[gsl] * SCALE).reshape(GH, P).T
        scl[:, 56:60] = bk[gsl].reshape(GH, P).T
        xw[r0 + XW_V:r0 + XW_V + P, 0:4 * SCL_C] = scl.view(np.int8)
        wsl = slice((D // 2) * hb, (D // 2) * (hb + 1))
        w0 = q0 + QK_BBLK
        xqk[w0:w0 + D // 2] = _pack7(np.ascontiguousarray(w_i8t["q"][wsl, gsl]))
        xqk[w0 + D // 2:w0 + D] = _pack7(
            np.ascontiguousarray(w_i8t["k"][wsl, gsl])
        )
        xw[r0 + R_W:r0 + R_W + D // 2] = w_i8t["v"][wsl, gsl]
        # wo half: 8 partition-friendly [128, 512] tiles (lc, oc)
        w1 = r0 + R_W + D // 2
        for lc in range(2):
            for oc in range(4):
                a0 = GD * g + (GD // 2) * hb + lc * P
                xw[w1 + (lc * 4 + oc) * P:w1 + (lc * 4 + oc + 1) * P] = (
                    w_i8t["o"][a0:a0 + P, oc * 512:(oc + 1) * 512]
                )

    return {"xw": xw, "xqk": xqk}


KC_COLS = D // P  # 16 scale columns per tensor in scl


def _gather(results, bo_eff):
    yi8 = results["y"]                       # [B*S, YP+4] batch-major
    sc = yi8[:, YP:YP + 4].copy().view(np.float32)
    u = yi8[:, :YP].view(np.uint8)
    v = np.empty((B * S, D), np.float32)
    v7 = np.zeros((B * S, D // 8), np.uint8)
    for i in range(7):
        pi = u[:, i::7]
        v[:, i::8] = (((pi & 0x7F) ^ 0x40).astype(np.int16) - 64).astype(
            np.float32
        )
        v7 |= (pi >> 7).astype(np.uint8) << i
    v[:, 7::8] = ((v7 ^ 0x40).astype(np.int16) - 64).astype(np.float32)
    return (v * sc + bo_eff[None, :]).reshape(B, S, D)


def kernel(**inputs):
    global _last_in_maps
    bo = np.asarray(inputs["bo"], np.float32)
    bv = np.asarray(inputs["bv"], np.float32)
    Wo = np.asarray(inputs["Wo"], np.float32)
    bo_eff = bo + bv @ Wo.T   # V bias folds through softmax-normalized P
    arrs = _prep_inputs(inputs)
    _last_in_maps = arrs
    runner = _get_runner()
    results = runner(arrs)
    return _gather(results, bo_eff)
